# revision 1
# baseline (speedup 1.0000x reference)
"""Trainium2 Bass kernel for nn_LEAP_74371653697613 (GRU decoder w/ additive attention).

Structure exploited:
  - softmax(ctx_score + h.w_h + b) == softmax(ctx_score): attention weights are
    constant across decode steps -> context vector c computed once on device.
  - gi_t = W_ih @ [c; x_t] + b_ih is teacher-forced -> batched matmuls, precomputed.
  - logits don't feed back -> one big relu(H) @ out_w^T matmul at the end,
    vocab-sharded across the 8 cores (each core gets a 4096-row slice of out_w).
  - only W_hh @ h_t + gates is sequential (65 steps); it runs identically on all
    8 cores (replicated -> zero cross-core communication).

Per-step device schedule:
  4 col-tiled groups of fp32 matvec MMs (h chunks stationary, W_hh^T moving) +
  a one-hot K=65 matmul folding gi_rz[t] into the same PSUM accumulation;
  bridge PSUM rows {0,32,64,96} -> SBUF via 2 ACT + 2 DVE copies; compact to
  contiguous partitions with a selector matmul; gates on [4, 256] tiles;
  h_stat rebuilt with 2 PE transposes.
"""
import os
import sys
import numpy as np

for _p in ("/opt/trn_rl_repo", "/root/.axon_site/_ro/trn_rl_repo"):
    if os.path.isdir(_p) and _p not in sys.path:
        sys.path.insert(0, _p)

import concourse.bass as bass
import concourse.bacc as bacc
import concourse.tile as tile
import concourse.mybir as mybir
from concourse.bass_utils import run_bass_kernel_spmd
from concourse.masks import make_identity

F32 = mybir.dt.float32
BF16 = mybir.dt.bfloat16
AF = mybir.ActivationFunctionType
ALU = mybir.AluOpType
NP_BF16 = mybir.dt.np(BF16)

E = 1024          # emb dim
EC = 8            # E / 128 chunks
T = 65            # decode steps (1 SOS + 64)
L = 320           # context rows (128 + 64 + 128)
V0 = 32000
V = V0 + 2        # 32002
NCORES = 8
VP = 4096         # per-core padded vocab slice (8 * 4096 = 32768 >= 32002)
G = 4             # col-tile groups
RG = 768          # region width per group (3 gates x 256)

_CACHE = {}


def _region_rows(w):
    """Reorder gate rows [3072, X] into region order: region (j, g*256+mm)
    <- row g*1024 + j*256 + mm."""
    x = w.reshape(3, 4, 256, -1)                 # g, j, mm, rest
    return np.ascontiguousarray(np.transpose(x, (1, 0, 2, 3))).reshape(3072, -1)


def _arrange_w(w):
    """[3072, 1024] -> [128, 8*4*768]: out[p, ((c*4)+j)*768 + g*256+mm]
    = w[g*1024 + j*256 + mm, c*128 + p]."""
    x = w.reshape(3, 4, 256, EC, 128)            # g, j, mm, c, p
    x = np.transpose(x, (4, 3, 1, 0, 2))         # p, c, j, g, mm
    return np.ascontiguousarray(x).reshape(128, EC * G * RG)


def _bias_tall(b_rzn):
    """[3072] bias in gate order -> [128, 768] with row 32j = region j."""
    x = b_rzn.reshape(3, 4, 256)                 # g, j, mm
    x = np.transpose(x, (1, 0, 2)).reshape(4, RG)  # j, (g mm)
    out = np.zeros((128, RG), np.float32)
    out[::32, :] = x
    return out


def build_program(rec_steps=T, do_final=True, do_gi=True, do_ph1=True, do_gic=True, do_wxdma=True):
    nc = bacc.Bacc("TRN2", target_bir_lowering=False, debug=False, num_devices=NCORES)

    ctx_d = nc.dram_tensor("ctx", [L, E], F32, kind="ExternalInput").ap()
    decx_d = nc.dram_tensor("decx", [T, E], F32, kind="ExternalInput").ap()
    we_d = nc.dram_tensor("we", [1, E], F32, kind="ExternalInput").ap()
    whh_d = nc.dram_tensor("whh", [128, EC * G * RG], F32, kind="ExternalInput").ap()
    wc_d = nc.dram_tensor("wc", [128, EC * G * RG], BF16, kind="ExternalInput").ap()
    wx_d = nc.dram_tensor("wx", [128, EC * G * RG], BF16, kind="ExternalInput").ap()
    bias_d = nc.dram_tensor("bias", [128, RG], F32, kind="ExternalInput").ap()
    owt_d = nc.dram_tensor("owt", [128, EC * VP], BF16, kind="ExternalInput").ap()
    outb_d = nc.dram_tensor("outb", [1, VP], F32, kind="ExternalInput").ap()
    out_d = nc.dram_tensor("out", [T, VP], F32, kind="ExternalOutput").ap()

    with tile.TileContext(nc) as tc:
        with tc.tile_pool(name="persist", bufs=1) as pp:
            # ---------- persistent constants ----------
            whh = pp.tile([128, EC * G * RG], F32)
            nc.sync.dma_start(whh[:], whh_d[:])

            ident = pp.tile([128, 128], F32)
            make_identity(nc, ident[:])
            ident_bf = pp.tile([128, 128], BF16)
            nc.vector.tensor_copy(ident_bf[:], ident[:])

            sel4 = pp.tile([128, 4], F32)          # sel4[k, m] = 1[k == 32m]
            nc.gpsimd.memset(sel4[:], 0.0)
            nc.gpsimd.affine_select(out=sel4[:], in_=sel4[:], compare_op=ALU.not_equal,
                                    fill=1.0, base=0, pattern=[[-32, 4]],
                                    channel_multiplier=1)

            ones_tall = pp.tile([128, T], F32)
            nc.gpsimd.memset(ones_tall[:], 1.0)
            ones_col = pp.tile([128, 1], F32)
            nc.gpsimd.memset(ones_col[:], 1.0)
            ones_row = pp.tile([1, 128], F32)
            nc.gpsimd.memset(ones_row[:], 1.0)

            bias_tall = pp.tile([128, RG], F32)    # (b_ih + b_hh) rz | b_ih n, rows at 32j
            nc.sync.dma_start(bias_tall[:], bias_d[:])

            girz = pp.tile([T, G, 512], BF16)      # gi rz-part, partition = t
            gin = pp.tile([4, T * 256], BF16)      # gi n-part, [j, t*256+m]
            gic_tall = pp.tile([128, RG], F32)     # const part of gi, rows at 32j
            ht_full = pp.tile([128, EC * T], BF16)  # [p, c, t] = relu(h_t[c*128+p])
            h_stat = pp.tile([128, EC], F32)
            h_rows = pp.tile([4, 256], F32)
            cT_bf = pp.tile([128, EC], BF16)
            dxT_bf = pp.tile([128, EC, T], BF16)
            # bridge buffers: only rows {32j} are written per step, but the
            # selector matmul streams all 128 partitions -> zero-init once so
            # unwritten rows are 0.0 (not stale/NaN bytes).
            gtallA = pp.tile([128, RG], F32)
            gtallB = pp.tile([128, RG], F32)
            nc.gpsimd.memset(gtallA[:], 0.0)
            nc.gpsimd.memset(gtallB[:], 0.0)

            # h0 = dec_emb[SOS] = decx row 0, in both layouts
            nc.sync.dma_start(h_stat[:], decx_d[0:1, :].rearrange("o (c p) -> (o p) c", p=128))
            nc.sync.dma_start(h_rows[:], decx_d[0:1, :].rearrange("o (j m) -> (o j) m", j=4))

            # ---------- phase 1: attention (constant across steps) ----------
            if do_ph1:
                with tc.tile_pool(name="ph1", bufs=1) as p1, \
                     tc.tile_pool(name="ph1ps", bufs=1, space="PSUM") as p1ps:
                    we_sb = p1.tile([1, E], F32)
                    nc.sync.dma_start(we_sb[:], we_d[:])
                    rows3 = (128, 128, 64)
                    ctx_sb = []
                    for i, rows in enumerate(rows3):
                        t_ = p1.tile([128, E], F32, tag=f"ctx{i}")
                        nc.sync.dma_start(t_[:rows, :], ctx_d[128 * i:128 * i + rows, :])
                        ctx_sb.append(t_)
                    decx_sb = p1.tile([T, E], F32)
                    nc.sync.dma_start(decx_sb[:], decx_d[:])

                    # replicate w_e across partitions via K=1 matmul
                    werep_ps = p1ps.tile([128, E], F32, space="PSUM")
                    for half in range(2):
                        nc.tensor.matmul(werep_ps[:, 512 * half:512 * (half + 1)],
                                         lhsT=ones_row[:1, :],
                                         rhs=we_sb[:1, 512 * half:512 * (half + 1)],
                                         start=True, stop=True)
                    werep = p1.tile([128, E], F32)
                    nc.vector.tensor_copy(werep[:], werep_ps[:])

                    scratch = p1.tile([128, E], F32)
                    escore = [p1.tile([128, 1], F32, tag=f"esc{i}", name=f"esc{i}")
                              for i in range(3)]
                    for i, rows in enumerate(rows3):
                        sc = p1.tile([128, 1], F32, tag=f"sc{i}")
                        nc.vector.tensor_tensor(out=scratch[:rows, :],
                                                in0=ctx_sb[i][:rows, :],
                                                in1=werep[:rows, :], op=ALU.mult)
                        nc.vector.tensor_reduce(out=sc[:rows, :], in_=scratch[:rows, :],
                                                axis=mybir.AxisListType.X, op=ALU.add)
                        nc.scalar.activation(escore[i][:rows, :], sc[:rows, :], AF.Exp)
                    ssum_ps = p1ps.tile([1, 1], F32, space="PSUM")
                    for i, rows in enumerate(rows3):
                        nc.tensor.matmul(ssum_ps[:1, :1], lhsT=escore[i][:rows, :1],
                                         rhs=ones_col[:rows, :1],
                                         start=(i == 0), stop=(i == 2))
                    rsum = p1.tile([1, 1], F32)
                    nc.vector.reciprocal(rsum[:], ssum_ps[:1, :1])

                    cun_ps = p1ps.tile([1, E], F32, space="PSUM")
                    for half in range(2):
                        for i, rows in enumerate(rows3):
                            nc.tensor.matmul(cun_ps[:1, 512 * half:512 * (half + 1)],
                                             lhsT=escore[i][:rows, :1],
                                             rhs=ctx_sb[i][:rows, 512 * half:512 * (half + 1)],
                                             start=(i == 0), stop=(i == 2))
                    c_sb = p1.tile([1, E], F32)
                    nc.vector.tensor_scalar_mul(c_sb[:], cun_ps[:1, :], rsum[:1, :1])

                    # c^T [128, 8] bf16 via PE transposes
                    cT_ps = p1ps.tile([128, EC], F32, space="PSUM")
                    for k in range(EC):
                        nc.tensor.transpose(out=cT_ps[:, k:k + 1],
                                            in_=c_sb[:1, 128 * k:128 * (k + 1)],
                                            identity=ident[:1, :1])
                    nc.vector.tensor_copy(cT_bf[:], cT_ps[:])

                    # dec_x^T [128, 8, 65] bf16 via PE transposes
                    dxT_ps = p1ps.tile([128, T], F32, space="PSUM")
                    for k in range(EC):
                        nc.tensor.transpose(out=dxT_ps[:, :],
                                            in_=decx_sb[:T, 128 * k:128 * (k + 1)],
                                            identity=ident[:T, :T])
                        nc.vector.tensor_copy(dxT_bf[:, k, :], dxT_ps[:, :])

            # ---------- phase 2: gic = W_ih[:, :E] @ c + biases ----------
            with tc.tile_pool(name="pwc", bufs=1) as pwc, \
                 tc.tile_pool(name="pwcps", bufs=1, space="PSUM") as pwcps:
                wc_sb = pwc.tile([128, EC * G * RG], BF16)
                nc.sync.dma_start(wc_sb[:], wc_d[:])
                wcv = wc_sb[:].rearrange("p (c j m) -> p c j m", c=EC, j=G)
                gic_ps = pwcps.tile([128, 1024], F32, space="PSUM")
                for j in range(G if do_gic else 0):
                    for c in range(EC):
                        nc.tensor.matmul(gic_ps[32 * j:32 * j + 1, 0:512],
                                         lhsT=cT_bf[:, c:c + 1], rhs=wcv[:, c, j, 0:512],
                                         start=(c == 0), stop=False,
                                         tile_position=(0, 32 * j))
                        nc.tensor.matmul(gic_ps[32 * j:32 * j + 1, 512:768],
                                         lhsT=cT_bf[:, c:c + 1], rhs=wcv[:, c, j, 512:768],
                                         start=(c == 0), stop=False,
                                         tile_position=(0, 32 * j))
                    # + (b_ih + b_hh)_rz | b_ih_n  via K=1 matmul
                    nc.tensor.matmul(gic_ps[32 * j:32 * j + 1, 0:512],
                                     lhsT=ones_tall[32 * j:32 * j + 1, 0:1],
                                     rhs=bias_tall[32 * j:32 * j + 1, 0:512],
                                     start=False, stop=True,
                                     tile_position=(32 * j, 32 * j))
                    nc.tensor.matmul(gic_ps[32 * j:32 * j + 1, 512:768],
                                     lhsT=ones_tall[32 * j:32 * j + 1, 0:1],
                                     rhs=bias_tall[32 * j:32 * j + 1, 512:768],
                                     start=False, stop=True,
                                     tile_position=(32 * j, 32 * j))
                for j in range(G if do_gic else 0):
                    if j % 2 == 0:
                        nc.scalar.copy(gic_tall[32 * j:32 * j + 1, :],
                                       gic_ps[32 * j:32 * j + 1, 0:RG])
                    else:
                        nc.vector.tensor_copy(gic_tall[32 * j:32 * j + 1, :],
                                              gic_ps[32 * j:32 * j + 1, 0:RG])

            # ---------- phase 3: gi[t] = gic + W_ih[:, E:] @ x_t ----------
            with tc.tile_pool(name="pwx", bufs=1) as pwx, \
                 tc.tile_pool(name="pwxps", bufs=2, space="PSUM") as pwxps:
                wx_sb = pwx.tile([128, EC * G * RG], BF16)
                if do_wxdma:
                    nc.sync.dma_start(wx_sb[:], wx_d[:])
                wxv = wx_sb[:].rearrange("p (c j m) -> p c j m", c=EC, j=G)
                for j in range(G if do_gi else 0):
                    rz_ps = pwxps.tile([T, 512], F32, space="PSUM", tag="girz")
                    for c in range(EC):
                        nc.tensor.matmul(rz_ps[:T, :], lhsT=dxT_bf[:, c, :],
                                         rhs=wxv[:, c, j, 0:512],
                                         start=(c == 0), stop=False)
                    nc.tensor.matmul(rz_ps[:T, :],
                                     lhsT=ones_tall[32 * j:32 * j + 1, :T],
                                     rhs=gic_tall[32 * j:32 * j + 1, 0:512],
                                     start=False, stop=True,
                                     tile_position=(32 * j, 0))
                    nc.vector.tensor_copy(girz[:, j, :], rz_ps[:T, :])

                    n_ps = pwxps.tile([T, 256], F32, space="PSUM", tag="gin")
                    for c in range(EC):
                        nc.tensor.matmul(n_ps[:T, :], lhsT=dxT_bf[:, c, :],
                                         rhs=wxv[:, c, j, 512:768],
                                         start=(c == 0), stop=False)
                    nc.tensor.matmul(n_ps[:T, :],
                                     lhsT=ones_tall[32 * j:32 * j + 1, :T],
                                     rhs=gic_tall[32 * j:32 * j + 1, 512:768],
                                     start=False, stop=True,
                                     tile_position=(32 * j, 0))
                    nbf = pwx.tile([T, 256], BF16, tag="ginbf")
                    nc.vector.tensor_copy(nbf[:], n_ps[:T, :])
                    nc.sync.dma_start(gin[j:j + 1, :], nbf[:T, :])

            # ---------- phase 4: the 65-step recurrence ----------
            whhv = whh[:].rearrange("p (c j m) -> p c j m", c=EC, j=G)
            htv4 = ht_full[:].rearrange("p (j par tt) -> p par j tt", j=4, par=2)
            with tc.tile_pool(name="rec", bufs=2) as pr, \
                 tc.tile_pool(name="recps_g", bufs=2, space="PSUM") as prg, \
                 tc.tile_pool(name="recps_s", bufs=1, space="PSUM") as prs:
                for t in range(rec_steps):
                    psg = prg.tile([128, 1024], F32, space="PSUM", tag="psg")
                    for j in range(G):
                        nc.tensor.matmul(psg[32 * j:32 * j + 1, 0:512],
                                         lhsT=ident_bf[:T, t:t + 1], rhs=girz[:T, j, :],
                                         start=True, stop=False, tile_position=(0, 32 * j))
                        for c in range(EC):
                            nc.tensor.matmul(psg[32 * j:32 * j + 1, 0:512],
                                             lhsT=h_stat[:, c:c + 1], rhs=whhv[:, c, j, 0:512],
                                             start=False, stop=(c == EC - 1),
                                             tile_position=(0, 32 * j))
                            nc.tensor.matmul(psg[32 * j:32 * j + 1, 512:768],
                                             lhsT=h_stat[:, c:c + 1], rhs=whhv[:, c, j, 512:768],
                                             start=(c == 0), stop=(c == EC - 1),
                                             tile_position=(0, 32 * j))
                    gtall = gtallA if t % 2 == 0 else gtallB
                    for j in range(G):
                        if j % 2 == 0:
                            nc.scalar.copy(gtall[32 * j:32 * j + 1, :],
                                           psg[32 * j:32 * j + 1, 0:RG])
                        else:
                            nc.vector.tensor_copy(gtall[32 * j:32 * j + 1, :],
                                                  psg[32 * j:32 * j + 1, 0:RG])
                    gcmp = prs.tile([4, 1024], F32, space="PSUM", tag="gcmp")
                    nc.tensor.matmul(gcmp[:4, 0:512], lhsT=sel4[:], rhs=gtall[:, 0:512],
                                     start=True, stop=True)
                    nc.tensor.matmul(gcmp[:4, 512:768], lhsT=sel4[:], rhs=gtall[:, 512:768],
                                     start=True, stop=True)
                    rz = pr.tile([4, 512], F32, tag="rz")
                    nc.scalar.activation(rz[:], gcmp[:4, 0:512], AF.Sigmoid)
                    t1 = pr.tile([4, 256], F32, tag="t1")
                    nc.vector.tensor_tensor(out=t1[:], in0=rz[:, 0:256],
                                            in1=gcmp[:4, 512:768], op=ALU.mult)
                    npre = pr.tile([4, 256], F32, tag="npre")
                    nc.vector.tensor_tensor(out=npre[:], in0=t1[:],
                                            in1=gin[:4, 256 * t:256 * (t + 1)], op=ALU.add)
                    n_g = pr.tile([4, 256], F32, tag="n_g")
                    nc.scalar.activation(n_g[:], npre[:], AF.Tanh)
                    d_g = pr.tile([4, 256], F32, tag="d_g")
                    nc.vector.tensor_tensor(out=d_g[:], in0=h_rows[:], in1=n_g[:],
                                            op=ALU.subtract)
                    zd = pr.tile([4, 256], F32, tag="zd")
                    nc.vector.tensor_tensor(out=zd[:], in0=rz[:, 256:512], in1=d_g[:],
                                            op=ALU.mult)
                    nc.vector.tensor_tensor(out=h_rows[:], in0=n_g[:], in1=zd[:],
                                            op=ALU.add)
                    hT_ps = prs.tile([128, 8], F32, space="PSUM", tag="hT")
                    nc.tensor.transpose(out=hT_ps[:, 0:4], in_=h_rows[:4, 0:128],
                                        identity=ident[:4, :4])
                    nc.tensor.transpose(out=hT_ps[:, 4:8], in_=h_rows[:4, 128:256],
                                        identity=ident[:4, :4])
                    # h_stat[:, 2j+par] <- hT_ps[:, par*4+j]
                    nc.vector.tensor_copy(
                        h_stat[:].rearrange("p (j par) -> p par j", par=2),
                        hT_ps[:, :].rearrange("p (par j) -> p par j", j=4))
                    # ht_full[p, (2j+par)*T + t] = relu(hT_ps[p, par*4+j])
                    nc.scalar.activation(
                        htv4[:, :, :, t:t + 1],
                        hT_ps[:, :].rearrange("p (par j) -> p par j", j=4).unsqueeze(3),
                        AF.Relu)

            # ---------- phase 5: logits = relu(H) @ out_w^T + out_b ----------
            owtv = owt_d.rearrange("p (c v) -> p c v", c=EC)
            htv = ht_full[:].rearrange("p (c tt) -> p c tt", c=EC)
            if not do_final:
                nc.sync.dma_start(out_d[0:T, 0:T], ones_tall[:T, :T])
            with tc.tile_pool(name="fin", bufs=2) as pf, \
                 tc.tile_pool(name="finps", bufs=2, space="PSUM") as pfps:
                outb_sb = pf.tile([1, VP], F32, tag="outb")
                if do_final:
                    nc.sync.dma_start(outb_sb[:], outb_d[:])
                for vb in range(VP // 512 if do_final else 0):
                    wchunk = pf.tile([128, EC, 512], BF16, tag="wchunk")
                    nc.sync.dma_start(wchunk[:], owtv[:, :, 512 * vb:512 * (vb + 1)])
                    ops = pfps.tile([T, 512], F32, space="PSUM", tag="ops")
                    for c in range(EC):
                        nc.tensor.matmul(ops[:T, :], lhsT=htv[:, c, :],
                                         rhs=wchunk[:, c, :],
                                         start=(c == 0), stop=False)
                    nc.tensor.matmul(ops[:T, :], lhsT=ones_tall[:1, :T],
                                     rhs=outb_sb[:1, 512 * vb:512 * (vb + 1)],
                                     start=False, stop=True)
                    osb = pf.tile([T, 512], F32, tag="osb")
                    nc.vector.tensor_copy(osb[:], ops[:T, :])
                    nc.sync.dma_start(out_d[:, 512 * vb:512 * (vb + 1)], osb[:])

    nc.compile()
    return nc


def _prep_inputs(inp):
    idx_enc = np.concatenate([inp["input_diagnosis"], inp["input_procedure"],
                              inp["input_medicine"]]).astype(np.int64)
    tokens = np.concatenate([np.array([V0], np.int64),
                             inp["dec_tokens"].astype(np.int64)])
    enc_emb = np.asarray(inp["enc_emb"], np.float32)
    dec_emb = np.asarray(inp["dec_emb"], np.float32)

    ctx = np.ascontiguousarray(enc_emb[idx_enc])                       # [320, 1024]
    decx = np.ascontiguousarray(dec_emb[tokens])                       # [65, 1024]
    we = np.ascontiguousarray(np.asarray(inp["attn_w"], np.float32)[0, E:]).reshape(1, E)

    w_ih = np.asarray(inp["gru_w_ih"], np.float32)                     # [3072, 2048]
    w_hh = np.asarray(inp["gru_w_hh"], np.float32)                     # [3072, 1024]
    b_ih = np.asarray(inp["gru_b_ih"], np.float32)
    b_hh = np.asarray(inp["gru_b_hh"], np.float32)
    assert not np.any(b_hh[2 * E:]), "nonzero b_hh n-gate not supported on device"

    whh_arr = _arrange_w(w_hh)                                         # [128, 24576] f32
    wc_arr = _arrange_w(np.ascontiguousarray(w_ih[:, :E])).astype(NP_BF16)
    wx_arr = _arrange_w(np.ascontiguousarray(w_ih[:, E:])).astype(NP_BF16)
    bias = b_ih.copy()
    bias[:2 * E] += b_hh[:2 * E]
    bias_arr = _bias_tall(bias)                                        # [128, 768] f32

    out_w = np.asarray(inp["out_w"], np.float32)
    out_b = np.asarray(inp["out_b"], np.float32)
    owp = np.zeros((NCORES * VP, E), np.float32)
    owp[:V] = out_w
    obp = np.zeros(NCORES * VP, np.float32)
    obp[:V] = out_b

    base = {"ctx": ctx, "decx": decx, "we": we, "whh": whh_arr,
            "wc": wc_arr, "wx": wx_arr, "bias": bias_arr}
    in_maps = []
    for i in range(NCORES):
        s = owp[i * VP:(i + 1) * VP]                                   # [4096, 1024]
        owt = np.ascontiguousarray(
            s.reshape(VP, EC, 128).transpose(2, 1, 0)).astype(NP_BF16).reshape(128, EC * VP)
        m = dict(base)
        m["owt"] = owt
        m["outb"] = np.ascontiguousarray(obp[i * VP:(i + 1) * VP]).reshape(1, VP)
        in_maps.append(m)
    return in_maps


def kernel(**inputs):
    if "nc" not in _CACHE:
        _CACHE["nc"] = build_program()
    nc = _CACHE["nc"]
    in_maps = _prep_inputs({k: np.asarray(v) for k, v in inputs.items()})
    res = run_bass_kernel_spmd(nc, in_maps, core_ids=list(range(NCORES)))
    slices = [res.results[i]["out"] for i in range(NCORES)]            # each [65, 4096]
    logits = np.concatenate(slices, axis=1)[:, :V]
    return np.ascontiguousarray(logits.astype(np.float32))



# revision 2
# speedup vs baseline: 3.6822x; 3.6822x over previous
"""Trainium2 Bass kernel for nn_LEAP_74371653697613 (GRU decoder w/ additive attention).

v2: Picard-sweep formulation of the recurrence.

Structure exploited (on top of the v1 observations):
  - softmax(ctx_score + h.w_h) == softmax(ctx_score): attention weights constant
    across steps -> context c computed once.
  - gi_t = W_ih @ [c; x_t] + b_ih precomputed batched (teacher forcing).
  - The h-recurrence h_t = (1-z_t) n_t + z_t h_{t-1} is solved by fixed-point
    iteration over the WHOLE sequence: each sweep computes gh = W_hh @ H_prev as
    ONE batched [65,3072] matmul (W_hh streamed once per sweep instead of once
    per step), gates batched, then the exact sequential structure is restored by
    the DVE prefix-scan  state = (z_t * state) - (z_t-1)*n_t  per hidden chunk.
    Contraction ~0.3x/sweep; KS sweeps reach the bf16 noise floor (~2.6e-3).
  - logits = relu(H) @ out_w^T batched at the end, vocab-sharded 8 ways.

Per-sweep schedule: rz-pass (4 regions x [65,512] PSUM: identity-fold of gi_rz +
8 K-chunk matmuls) with sigmoids chasing region completion; n-pass (4 x [65,256])
with r*hn and +gi_n chasing; 16 PE transposes (z, npre) into [128, (c,t)] PSUM;
tanh + (z-1)*n fused STT; 8 tensor_tensor_scans; one strided copy shifts the scan
output into the next sweep's lhsT.
"""
import os
import sys
import numpy as np

for _p in ("/opt/trn_rl_repo", "/root/.axon_site/_ro/trn_rl_repo"):
    if os.path.isdir(_p) and _p not in sys.path:
        sys.path.insert(0, _p)

import concourse.bass as bass
import concourse.bacc as bacc
import concourse.tile as tile
import concourse.mybir as mybir
from concourse.bass_utils import run_bass_kernel_spmd
from concourse.masks import make_identity

F32 = mybir.dt.float32
BF16 = mybir.dt.bfloat16
AF = mybir.ActivationFunctionType
ALU = mybir.AluOpType
NP_BF16 = mybir.dt.np(BF16)

E = 1024          # emb dim
EC = 8            # E / 128 chunks
T = 65            # decode steps (1 SOS + 64)
L = 320           # context rows (128 + 64 + 128)
V0 = 32000
V = V0 + 2        # 32002
NCORES = 8
VP = 4096         # per-core padded vocab slice (8 * 4096 = 32768 >= 32002)
G = 4             # gate regions (each 256 hidden units x 3 gates)
RG = 768          # region width (3 gates x 256)
KS = 7            # Picard sweeps

_CACHE = {}


def _region_rows(w):
    x = w.reshape(3, 4, 256, -1)                 # g, j, mm, rest
    return np.ascontiguousarray(np.transpose(x, (1, 0, 2, 3))).reshape(3072, -1)


def _arrange_w(w):
    """[3072, 1024] -> [128, 8*4*768]: out[p, ((c*4)+j)*768 + g*256+mm]
    = w[g*1024 + j*256 + mm, c*128 + p]."""
    x = w.reshape(3, 4, 256, EC, 128)            # g, j, mm, c, p
    x = np.transpose(x, (4, 3, 1, 0, 2))         # p, c, j, g, mm
    return np.ascontiguousarray(x).reshape(128, EC * G * RG)


def _bias_tall(b_rzn):
    x = b_rzn.reshape(3, 4, 256)                 # g, j, mm
    x = np.transpose(x, (1, 0, 2)).reshape(4, RG)  # j, (g mm)
    out = np.zeros((128, RG), np.float32)
    out[::32, :] = x
    return out


def build_program(ksweeps=KS, do_final=True, outer_reps=1):
    nc = bacc.Bacc("TRN2", target_bir_lowering=False, debug=False, num_devices=NCORES)

    ctx_d = nc.dram_tensor("ctx", [L, E], F32, kind="ExternalInput").ap()
    decx_d = nc.dram_tensor("decx", [T, E], F32, kind="ExternalInput").ap()
    we_d = nc.dram_tensor("we", [1, E], F32, kind="ExternalInput").ap()
    whh_d = nc.dram_tensor("whh", [128, EC * G * RG], BF16, kind="ExternalInput").ap()
    wc_d = nc.dram_tensor("wc", [128, EC * G * RG], BF16, kind="ExternalInput").ap()
    wx_d = nc.dram_tensor("wx", [128, EC * G * RG], BF16, kind="ExternalInput").ap()
    bias_d = nc.dram_tensor("bias", [128, RG], F32, kind="ExternalInput").ap()
    owt_d = nc.dram_tensor("owt", [128, EC * VP], BF16, kind="ExternalInput").ap()
    outb_d = nc.dram_tensor("outb", [1, VP], F32, kind="ExternalInput").ap()
    out_d = nc.dram_tensor("out", [T, VP], F32, kind="ExternalOutput").ap()

    with tile.TileContext(nc) as tc:
        with tc.tile_pool(name="persist", bufs=1) as pp:
            # ---------- persistent constants ----------
            whh = pp.tile([128, EC * G * RG], BF16)
            nc.sync.dma_start(whh[:], whh_d[:])

            ident = pp.tile([128, 128], F32)
            make_identity(nc, ident[:])

            ones_tall = pp.tile([128, T], F32)
            nc.gpsimd.memset(ones_tall[:], 1.0)

            bias_tall = pp.tile([128, RG], F32)
            nc.sync.dma_start(bias_tall[:], bias_d[:])

            girz = pp.tile([T, G, 512], F32)       # gi rz-part, partition = t
            gin65 = pp.tile([T, 1024], F32)        # gi n-part, partition = t
            gic_tall = pp.tile([128, RG], F32)     # const part of gi, rows at 32j
            h_stat = pp.tile([128, EC], F32)       # h0 chunks (scan initial)
            hprevT = pp.tile([128, EC * T], BF16)  # lhsT: (c,t) -> h_{t-1}[c*128+p]
            hscan = pp.tile([128, EC * T], BF16)   # scan out: (c,t) -> h_t
            htf = pp.tile([128, EC * T], BF16)     # relu(hscan)
            cT_bf = pp.tile([128, EC], BF16)
            dxT_bf = pp.tile([128, EC, T], BF16)

            # h0 = dec_emb[SOS] = decx row 0 in stationary layout
            nc.sync.dma_start(h_stat[:], decx_d[0:1, :].rearrange("o (c p) -> (o p) c", p=128))

            # ---------- phase 1: attention (constant across steps) ----------
            with tc.tile_pool(name="ph1", bufs=1) as p1, \
                 tc.tile_pool(name="ph1ps", bufs=1, space="PSUM") as p1ps:
                ones_col = p1.tile([128, 1], F32)
                nc.gpsimd.memset(ones_col[:], 1.0)
                ones_row = p1.tile([1, 128], F32)
                nc.gpsimd.memset(ones_row[:], 1.0)
                we_sb = p1.tile([1, E], F32)
                nc.sync.dma_start(we_sb[:], we_d[:])
                rows3 = (128, 128, 64)
                ctx_sb = []
                for i, rows in enumerate(rows3):
                    t_ = p1.tile([128, E], F32, tag=f"ctx{i}")
                    nc.sync.dma_start(t_[:rows, :], ctx_d[128 * i:128 * i + rows, :])
                    ctx_sb.append(t_)
                decx_sb = p1.tile([T, E], F32)
                nc.sync.dma_start(decx_sb[:], decx_d[:])

                werep_ps = p1ps.tile([128, E], F32, space="PSUM")
                for half in range(2):
                    nc.tensor.matmul(werep_ps[:, 512 * half:512 * (half + 1)],
                                     lhsT=ones_row[:1, :],
                                     rhs=we_sb[:1, 512 * half:512 * (half + 1)],
                                     start=True, stop=True)
                werep = p1.tile([128, E], F32)
                nc.vector.tensor_copy(werep[:], werep_ps[:])

                scratch = p1.tile([128, E], F32)
                escore = [p1.tile([128, 1], F32, tag=f"esc{i}", name=f"esc{i}")
                          for i in range(3)]
                for i, rows in enumerate(rows3):
                    sc = p1.tile([128, 1], F32, tag=f"sc{i}")
                    nc.vector.tensor_tensor(out=scratch[:rows, :],
                                            in0=ctx_sb[i][:rows, :],
                                            in1=werep[:rows, :], op=ALU.mult)
                    nc.vector.tensor_reduce(out=sc[:rows, :], in_=scratch[:rows, :],
                                            axis=mybir.AxisListType.X, op=ALU.add)
                    nc.scalar.activation(escore[i][:rows, :], sc[:rows, :], AF.Exp)
                ssum_ps = p1ps.tile([1, 1], F32, space="PSUM")
                for i, rows in enumerate(rows3):
                    nc.tensor.matmul(ssum_ps[:1, :1], lhsT=escore[i][:rows, :1],
                                     rhs=ones_col[:rows, :1],
                                     start=(i == 0), stop=(i == 2))
                rsum = p1.tile([1, 1], F32)
                nc.vector.reciprocal(rsum[:], ssum_ps[:1, :1])

                cun_ps = p1ps.tile([1, E], F32, space="PSUM")
                for half in range(2):
                    for i, rows in enumerate(rows3):
                        nc.tensor.matmul(cun_ps[:1, 512 * half:512 * (half + 1)],
                                         lhsT=escore[i][:rows, :1],
                                         rhs=ctx_sb[i][:rows, 512 * half:512 * (half + 1)],
                                         start=(i == 0), stop=(i == 2))
                c_sb = p1.tile([1, E], F32)
                nc.vector.tensor_scalar_mul(c_sb[:], cun_ps[:1, :], rsum[:1, :1])

                cT_ps = p1ps.tile([128, EC], F32, space="PSUM")
                for k in range(EC):
                    nc.tensor.transpose(out=cT_ps[:, k:k + 1],
                                        in_=c_sb[:1, 128 * k:128 * (k + 1)],
                                        identity=ident[:1, :1])
                nc.vector.tensor_copy(cT_bf[:], cT_ps[:])

                dxT_ps = p1ps.tile([128, T], F32, space="PSUM")
                for k in range(EC):
                    nc.tensor.transpose(out=dxT_ps[:, :],
                                        in_=decx_sb[:T, 128 * k:128 * (k + 1)],
                                        identity=ident[:T, :T])
                    nc.vector.tensor_copy(dxT_bf[:, k, :], dxT_ps[:, :])

            # ---------- phase 2: gic = W_ih[:, :E] @ c + biases ----------
            with tc.tile_pool(name="pwc", bufs=1) as pwc, \
                 tc.tile_pool(name="pwcps", bufs=1, space="PSUM") as pwcps:
                wc_sb = pwc.tile([128, EC * G * RG], BF16)
                nc.sync.dma_start(wc_sb[:], wc_d[:])
                wcv = wc_sb[:].rearrange("p (c j m) -> p c j m", c=EC, j=G)
                gic_ps = pwcps.tile([128, 1024], F32, space="PSUM")
                for j in range(G):
                    for c in range(EC):
                        nc.tensor.matmul(gic_ps[32 * j:32 * j + 1, 0:512],
                                         lhsT=cT_bf[:, c:c + 1], rhs=wcv[:, c, j, 0:512],
                                         start=(c == 0), stop=False,
                                         tile_position=(0, 32 * j))
                        nc.tensor.matmul(gic_ps[32 * j:32 * j + 1, 512:768],
                                         lhsT=cT_bf[:, c:c + 1], rhs=wcv[:, c, j, 512:768],
                                         start=(c == 0), stop=False,
                                         tile_position=(0, 32 * j))
                    nc.tensor.matmul(gic_ps[32 * j:32 * j + 1, 0:512],
                                     lhsT=ones_tall[32 * j:32 * j + 1, 0:1],
                                     rhs=bias_tall[32 * j:32 * j + 1, 0:512],
                                     start=False, stop=True,
                                     tile_position=(32 * j, 32 * j))
                    nc.tensor.matmul(gic_ps[32 * j:32 * j + 1, 512:768],
                                     lhsT=ones_tall[32 * j:32 * j + 1, 0:1],
                                     rhs=bias_tall[32 * j:32 * j + 1, 512:768],
                                     start=False, stop=True,
                                     tile_position=(32 * j, 32 * j))
                for j in range(G):
                    if j % 2 == 0:
                        nc.scalar.copy(gic_tall[32 * j:32 * j + 1, :],
                                       gic_ps[32 * j:32 * j + 1, 0:RG])
                    else:
                        nc.vector.tensor_copy(gic_tall[32 * j:32 * j + 1, :],
                                              gic_ps[32 * j:32 * j + 1, 0:RG])

            # ---------- phase 3: gi[t] = gic + W_ih[:, E:] @ x_t (batched) ----------
            with tc.tile_pool(name="pwx", bufs=1) as pwx, \
                 tc.tile_pool(name="pwxps", bufs=2, space="PSUM") as pwxps:
                wx_sb = pwx.tile([128, EC * G * RG], BF16)
                nc.sync.dma_start(wx_sb[:], wx_d[:])
                wxv = wx_sb[:].rearrange("p (c j m) -> p c j m", c=EC, j=G)
                for j in range(G):
                    rz_ps = pwxps.tile([T, 512], F32, space="PSUM", tag="girz")
                    for c in range(EC):
                        nc.tensor.matmul(rz_ps[:T, :], lhsT=dxT_bf[:, c, :],
                                         rhs=wxv[:, c, j, 0:512],
                                         start=(c == 0), stop=False)
                    nc.tensor.matmul(rz_ps[:T, :],
                                     lhsT=ones_tall[32 * j:32 * j + 1, :T],
                                     rhs=gic_tall[32 * j:32 * j + 1, 0:512],
                                     start=False, stop=True,
                                     tile_position=(32 * j, 0))
                    nc.vector.tensor_copy(girz[:, j, :], rz_ps[:T, :])

                    n_ps = pwxps.tile([T, 256], F32, space="PSUM", tag="gin")
                    for c in range(EC):
                        nc.tensor.matmul(n_ps[:T, :], lhsT=dxT_bf[:, c, :],
                                         rhs=wxv[:, c, j, 512:768],
                                         start=(c == 0), stop=False)
                    nc.tensor.matmul(n_ps[:T, :],
                                     lhsT=ones_tall[32 * j:32 * j + 1, :T],
                                     rhs=gic_tall[32 * j:32 * j + 1, 512:768],
                                     start=False, stop=True,
                                     tile_position=(32 * j, 0))
                    nc.vector.tensor_copy(gin65[:, 256 * j:256 * (j + 1)], n_ps[:T, :])

            # prefetch final-phase weights (issued after whh/wc/wx in program order)
            owt_sb = pp.tile([128, EC * VP], BF16)
            outb_sb = pp.tile([1, VP], F32)
            if do_final:
                nc.sync.dma_start(owt_sb[:], owt_d[:])
                nc.sync.dma_start(outb_sb[:], outb_d[:])

            # ---------- phase 4: Picard sweeps ----------
            whhv = whh[:].rearrange("p (c j m) -> p c j m", c=EC, j=G)
            hprevT_v = hprevT[:].rearrange("p (c t) -> p c t", c=EC)
            hscan_v = hscan[:].rearrange("p (c t) -> p c t", c=EC)

            # init H_prev[t] = h0 for all t
            for c in range(EC):
                nc.vector.tensor_scalar_mul(hprevT[:, c * T:(c + 1) * T],
                                            ones_tall[:, :T], h_stat[:, c:c + 1])

            with tc.tile_pool(name="sw", bufs=4) as psw, \
                 tc.tile_pool(name="swg", bufs=2) as psg2, \
                 tc.tile_pool(name="swps", bufs=2, space="PSUM") as pps1, \
                 tc.tile_pool(name="swpsT", bufs=1, space="PSUM") as pps2:
                for rep in range(outer_reps):
                    for k in range(ksweeps):
                        sgs, npres = [], []
                        # rz pass: 4 regions, gi fold + 8 K-chunks each
                        for j in range(G):
                            rz = pps1.tile([T, 512], F32, space="PSUM", tag="rz")
                            nc.tensor.matmul(rz[:T, :], lhsT=ident[:T, :T],
                                             rhs=girz[:, j, :], start=True, stop=False)
                            for c in range(EC):
                                nc.tensor.matmul(rz[:T, :],
                                                 lhsT=hprevT[:, c * T:(c + 1) * T],
                                                 rhs=whhv[:, c, j, 0:512],
                                                 start=False, stop=(c == EC - 1))
                            sg = psw.tile([T, 512], F32, tag="sg")
                            nc.scalar.activation(sg[:], rz[:T, :], AF.Sigmoid)
                            sgs.append(sg)
                        # n pass
                        for j in range(G):
                            nps_j = pps1.tile([T, 256], F32, space="PSUM", tag="n")
                            for c in range(EC):
                                nc.tensor.matmul(nps_j[:T, :],
                                                 lhsT=hprevT[:, c * T:(c + 1) * T],
                                                 rhs=whhv[:, c, j, 512:768],
                                                 start=(c == 0), stop=(c == EC - 1))
                            t1 = psg2.tile([T, 256], F32, tag="t1")
                            nc.vector.tensor_tensor(out=t1[:], in0=sgs[j][:, 0:256],
                                                    in1=nps_j[:T, :], op=ALU.mult)
                            npre = psw.tile([T, 256], F32, tag="npre")
                            nc.vector.tensor_tensor(out=npre[:], in0=t1[:],
                                                    in1=gin65[:, 256 * j:256 * (j + 1)],
                                                    op=ALU.add)
                            npres.append(npre)
                        # transposes into [128, (half, cc, t)] PSUM; chunk cc at
                        # column 512*(cc//4) + 65*(cc%4)
                        zT = pps2.tile([128, 1024], F32, space="PSUM", tag="zT")
                        npT = pps2.tile([128, 1024], F32, space="PSUM", tag="npT")
                        for cc in range(EC):
                            j, k2 = cc // 2, cc % 2
                            col = 512 * (cc // 4) + T * (cc % 4)
                            nc.tensor.transpose(out=zT[:, col:col + T],
                                                in_=sgs[j][:T, 256 + 128 * k2:256 + 128 * (k2 + 1)],
                                                identity=ident[:T, :T])
                            nc.tensor.transpose(out=npT[:, col:col + T],
                                                in_=npres[j][:T, 128 * k2:128 * (k2 + 1)],
                                                identity=ident[:T, :T])
                        nT = psg2.tile([128, 1024], F32, tag="nT")
                        wsb = psg2.tile([128, 1024], F32, tag="wsb")
                        for h in range(2):
                            s = slice(512 * h, 512 * h + 4 * T)
                            nc.scalar.activation(nT[:, s], npT[:, s], AF.Tanh)
                            # (z - 1) * n
                            nc.vector.scalar_tensor_tensor(out=wsb[:, s], in0=zT[:, s],
                                                           scalar=1.0, in1=nT[:, s],
                                                           op0=ALU.subtract, op1=ALU.mult)
                        for cc in range(EC):
                            col = 512 * (cc // 4) + T * (cc % 4)
                            # state = z*state - (z-1)*n
                            nc.vector.tensor_tensor_scan(
                                out=hscan[:, cc * T:(cc + 1) * T],
                                data0=zT[:, col:col + T], data1=wsb[:, col:col + T],
                                initial=h_stat[:, cc:cc + 1],
                                op0=ALU.mult, op1=ALU.subtract)
                        last = (k == ksweeps - 1) and (rep == outer_reps - 1)
                        if not last:
                            nc.vector.tensor_copy(hprevT_v[:, :, 1:T],
                                                  hscan_v[:, :, 0:T - 1])

            # ---------- phase 5: logits = relu(H) @ out_w^T + out_b ----------
            nc.scalar.activation(htf[:], hscan[:], AF.Relu)
            owtv = owt_sb[:].rearrange("p (c v) -> p c v", c=EC)
            htv = htf[:].rearrange("p (c t) -> p c t", c=EC)
            if not do_final:
                nc.sync.dma_start(out_d[0:T, 0:T], ones_tall[:T, :T])
            with tc.tile_pool(name="fin", bufs=2) as pf, \
                 tc.tile_pool(name="finps", bufs=2, space="PSUM") as pfps:
                for vb in range(VP // 512 if do_final else 0):
                    ops = pfps.tile([T, 512], F32, space="PSUM", tag="ops")
                    for c in range(EC):
                        nc.tensor.matmul(ops[:T, :], lhsT=htv[:, c, :],
                                         rhs=owtv[:, c, 512 * vb:512 * (vb + 1)],
                                         start=(c == 0), stop=False)
                    nc.tensor.matmul(ops[:T, :], lhsT=ones_tall[:1, :T],
                                     rhs=outb_sb[:1, 512 * vb:512 * (vb + 1)],
                                     start=False, stop=True)
                    osb = pf.tile([T, 512], F32, tag="osb")
                    if vb % 2 == 0:
                        nc.vector.tensor_copy(osb[:], ops[:T, :])
                    else:
                        nc.scalar.copy(osb[:], ops[:T, :])
                    nc.sync.dma_start(out_d[:, 512 * vb:512 * (vb + 1)], osb[:])

    nc.compile()
    return nc


def _prep_inputs(inp):
    idx_enc = np.concatenate([inp["input_diagnosis"], inp["input_procedure"],
                              inp["input_medicine"]]).astype(np.int64)
    tokens = np.concatenate([np.array([V0], np.int64),
                             inp["dec_tokens"].astype(np.int64)])
    enc_emb = np.asarray(inp["enc_emb"], np.float32)
    dec_emb = np.asarray(inp["dec_emb"], np.float32)

    ctx = np.ascontiguousarray(enc_emb[idx_enc])                       # [320, 1024]
    decx = np.ascontiguousarray(dec_emb[tokens])                       # [65, 1024]
    we = np.ascontiguousarray(np.asarray(inp["attn_w"], np.float32)[0, E:]).reshape(1, E)

    w_ih = np.asarray(inp["gru_w_ih"], np.float32)                     # [3072, 2048]
    w_hh = np.asarray(inp["gru_w_hh"], np.float32)                     # [3072, 1024]
    b_ih = np.asarray(inp["gru_b_ih"], np.float32)
    b_hh = np.asarray(inp["gru_b_hh"], np.float32)
    assert not np.any(b_hh[2 * E:]), "nonzero b_hh n-gate not supported on device"

    whh_arr = _arrange_w(w_hh).astype(NP_BF16)                         # [128, 24576]
    wc_arr = _arrange_w(np.ascontiguousarray(w_ih[:, :E])).astype(NP_BF16)
    wx_arr = _arrange_w(np.ascontiguousarray(w_ih[:, E:])).astype(NP_BF16)
    bias = b_ih.copy()
    bias[:2 * E] += b_hh[:2 * E]
    bias_arr = _bias_tall(bias)                                        # [128, 768] f32

    out_w = np.asarray(inp["out_w"], np.float32)
    out_b = np.asarray(inp["out_b"], np.float32)
    owp = np.zeros((NCORES * VP, E), np.float32)
    owp[:V] = out_w
    obp = np.zeros(NCORES * VP, np.float32)
    obp[:V] = out_b

    base = {"ctx": ctx, "decx": decx, "we": we, "whh": whh_arr,
            "wc": wc_arr, "wx": wx_arr, "bias": bias_arr}
    in_maps = []
    for i in range(NCORES):
        s = owp[i * VP:(i + 1) * VP]                                   # [4096, 1024]
        owt = np.ascontiguousarray(
            s.reshape(VP, EC, 128).transpose(2, 1, 0)).astype(NP_BF16).reshape(128, EC * VP)
        m = dict(base)
        m["owt"] = owt
        m["outb"] = np.ascontiguousarray(obp[i * VP:(i + 1) * VP]).reshape(1, VP)
        in_maps.append(m)
    return in_maps


def kernel(**inputs):
    if "nc" not in _CACHE:
        _CACHE["nc"] = build_program()
    nc = _CACHE["nc"]
    in_maps = _prep_inputs({k: np.asarray(v) for k, v in inputs.items()})
    res = run_bass_kernel_spmd(nc, in_maps, core_ids=list(range(NCORES)))
    slices = [res.results[i]["out"] for i in range(NCORES)]            # each [65, 4096]
    logits = np.concatenate(slices, axis=1)[:, :V]
    return np.ascontiguousarray(logits.astype(np.float32))


# revision 3
# speedup vs baseline: 27.8759x; 7.5704x over previous
"""Trainium2 Bass kernel for nn_LEAP_74371653697613 (GRU decoder w/ additive attention).

v2: Picard-sweep formulation of the recurrence.

Structure exploited:
  - softmax(ctx_score + h.w_h) == softmax(ctx_score): attention weights constant
    across steps -> context c computed once.
  - gi_t = W_ih @ [c; x_t] + b_ih precomputed batched (teacher forcing).
  - The h-recurrence h_t = (1-z_t) n_t + z_t h_{t-1} is solved by fixed-point
    iteration over the WHOLE sequence: each sweep computes gh = W_hh @ H_prev as
    ONE batched [65,3072] matmul (W_hh streamed once per sweep instead of once
    per step), gates batched, then the exact sequential structure is restored by
    the DVE prefix-scan  state = (z_t * state) - (z_t - 1)*n_t  per hidden chunk.
    Contraction ~0.3x/sweep; KS sweeps reach the bf16 noise floor (~2.6e-3 rel).
  - Sweep 1 has H_prev === h0, so its gh is a single col-tiled matvec W_hh @ h0
    broadcast across t (cheap) instead of the batched matmul.
  - logits = relu(H) @ out_w^T batched at the end, vocab-sharded 8 ways
    (each core owns a 4096-row slice of out_w); out_w prefetched during sweeps.

outer_reps repeats phases 2-5 inside one NEFF for slope-based timing; the
shipped kernel uses outer_reps=1.
"""
import os
import sys
import numpy as np

for _p in ("/opt/trn_rl_repo", "/root/.axon_site/_ro/trn_rl_repo"):
    if os.path.isdir(_p) and _p not in sys.path:
        sys.path.insert(0, _p)

import concourse.bass as bass
import concourse.bacc as bacc
import concourse.tile as tile
import concourse.mybir as mybir
from concourse.bass_utils import run_bass_kernel_spmd
from concourse.masks import make_identity

F32 = mybir.dt.float32
BF16 = mybir.dt.bfloat16
AF = mybir.ActivationFunctionType
ALU = mybir.AluOpType
NP_BF16 = mybir.dt.np(BF16)

E = 1024          # emb dim
EC = 8            # E / 128 chunks
T = 65            # decode steps (1 SOS + 64)
L = 320           # context rows (128 + 64 + 128)
V0 = 32000
V = V0 + 2        # 32002
NCORES = 8
VP = 4096         # per-core padded vocab slice (8 * 4096 = 32768 >= 32002)
G = 4             # gate regions (each 256 hidden units x 3 gates)
RG = 768          # region width (3 gates x 256)
CW = G * RG       # per-K-chunk weight width (3072)
KS = 6            # Picard sweeps (incl. the specialized first sweep)
TP = 66           # padded t-stride in hprevT: slot 0 = h0, slots 1..65 = scan out

_CACHE = {}


def _arrange_w(w):
    """[3072, 1024] -> [128, 8*4*768]: out[p, ((c*4)+j)*768 + g*256+mm]
    = w[g*1024 + j*256 + mm, c*128 + p]."""
    x = w.reshape(3, 4, 256, EC, 128)            # g, j, mm, c, p
    x = np.transpose(x, (4, 3, 1, 0, 2))         # p, c, j, g, mm
    return np.ascontiguousarray(x).reshape(128, EC * CW)


def _bias_tall(b_rzn):
    x = b_rzn.reshape(3, 4, 256)                 # g, j, mm
    x = np.transpose(x, (1, 0, 2)).reshape(4, RG)  # j, (g mm)
    out = np.zeros((128, RG), np.float32)
    out[::32, :] = x
    return out


def build_program(ksweeps=KS, do_final=True, outer_reps=1):
    nc = bacc.Bacc("TRN2", target_bir_lowering=False, debug=False, num_devices=NCORES)

    ctx_d = nc.dram_tensor("ctx", [L, E], F32, kind="ExternalInput").ap()
    decx_d = nc.dram_tensor("decx", [T, E], F32, kind="ExternalInput").ap()
    we_d = nc.dram_tensor("we", [1, E], F32, kind="ExternalInput").ap()
    whh_d = nc.dram_tensor("whh", [128, EC * CW], BF16, kind="ExternalInput").ap()
    wc_d = nc.dram_tensor("wc", [128, EC * CW], BF16, kind="ExternalInput").ap()
    wx_d = nc.dram_tensor("wx", [128, EC * CW], BF16, kind="ExternalInput").ap()
    bias_d = nc.dram_tensor("bias", [128, RG], F32, kind="ExternalInput").ap()
    owt_d = nc.dram_tensor("owt", [128, EC * VP], BF16, kind="ExternalInput").ap()
    outb_d = nc.dram_tensor("outb", [1, VP], F32, kind="ExternalInput").ap()
    out_d = nc.dram_tensor("out", [T, VP], F32, kind="ExternalOutput").ap()

    with tile.TileContext(nc) as tc:
        with tc.tile_pool(name="persist", bufs=1) as pp:
            # ---------- persistent constants ----------
            whh = pp.tile([128, EC * CW], BF16)
            for c in range(EC):
                nc.sync.dma_start(whh[:, c * CW:(c + 1) * CW],
                                  whh_d[:, c * CW:(c + 1) * CW])

            ident = pp.tile([128, 128], F32)
            make_identity(nc, ident[:])

            ones_tall = pp.tile([128, T], F32)
            nc.gpsimd.memset(ones_tall[:], 1.0)

            bias_tall = pp.tile([128, RG], F32)
            nc.sync.dma_start(bias_tall[:], bias_d[:])

            girz = pp.tile([T, G, 512], F32)       # gi rz-part, partition = t
            gin65 = pp.tile([T, 1024], F32)        # gi n-part, partition = t
            gic_tall = pp.tile([128, RG], F32)     # const part of gi, rows at 32j
            h_stat = pp.tile([128, EC], F32)       # h0 chunks (scan initial)
            h0bf = pp.tile([128, EC], BF16)
            gh0 = pp.tile([128, RG], F32)          # W_hh @ h0, region rows at 32j
            hprevT = pp.tile([128, EC * TP], BF16)  # slot (c,0)=h0; (c,1..65)=h_1..h_65
            htf = pp.tile([128, EC * T], BF16)      # relu(h_1..h_65)
            cT_bf = pp.tile([128, EC], BF16)
            dxT_bf = pp.tile([128, EC, T], BF16)

            # h0 = dec_emb[SOS] = decx row 0 in stationary layout
            nc.sync.dma_start(h_stat[:], decx_d[0:1, :].rearrange("o (c p) -> (o p) c", p=128))
            nc.vector.tensor_copy(h0bf[:], h_stat[:])

            whhv = whh[:].rearrange("p (c j m) -> p c j m", c=EC, j=G)
            hprevT_v = hprevT[:].rearrange("p (c t) -> p c t", c=EC)

            # ---------- phase 1: attention (constant across steps) ----------
            with tc.tile_pool(name="ph1", bufs=1) as p1, \
                 tc.tile_pool(name="ph1ps", bufs=1, space="PSUM") as p1ps:
                ones_col = p1.tile([128, 1], F32)
                nc.gpsimd.memset(ones_col[:], 1.0)
                ones_row = p1.tile([1, 128], F32)
                nc.gpsimd.memset(ones_row[:], 1.0)
                we_sb = p1.tile([1, E], F32)
                nc.sync.dma_start(we_sb[:], we_d[:])
                rows3 = (128, 128, 64)
                ctx_sb = []
                for i, rows in enumerate(rows3):
                    t_ = p1.tile([128, E], F32, tag=f"ctx{i}")
                    nc.sync.dma_start(t_[:rows, :], ctx_d[128 * i:128 * i + rows, :])
                    ctx_sb.append(t_)
                decx_sb = p1.tile([T, E], F32)
                nc.sync.dma_start(decx_sb[:], decx_d[:])

                werep_ps = p1ps.tile([128, E], F32, space="PSUM")
                for half in range(2):
                    nc.tensor.matmul(werep_ps[:, 512 * half:512 * (half + 1)],
                                     lhsT=ones_row[:1, :],
                                     rhs=we_sb[:1, 512 * half:512 * (half + 1)],
                                     start=True, stop=True)
                werep = p1.tile([128, E], F32)
                nc.vector.tensor_copy(werep[:], werep_ps[:])

                scratch = p1.tile([128, E], F32)
                escore = [p1.tile([128, 1], F32, tag=f"esc{i}", name=f"esc{i}")
                          for i in range(3)]
                for i, rows in enumerate(rows3):
                    sc = p1.tile([128, 1], F32, tag=f"sc{i}")
                    nc.vector.tensor_tensor(out=scratch[:rows, :],
                                            in0=ctx_sb[i][:rows, :],
                                            in1=werep[:rows, :], op=ALU.mult)
                    nc.vector.tensor_reduce(out=sc[:rows, :], in_=scratch[:rows, :],
                                            axis=mybir.AxisListType.X, op=ALU.add)
                    nc.scalar.activation(escore[i][:rows, :], sc[:rows, :], AF.Exp)
                ssum_ps = p1ps.tile([1, 1], F32, space="PSUM")
                for i, rows in enumerate(rows3):
                    nc.tensor.matmul(ssum_ps[:1, :1], lhsT=escore[i][:rows, :1],
                                     rhs=ones_col[:rows, :1],
                                     start=(i == 0), stop=(i == 2))
                rsum = p1.tile([1, 1], F32)
                nc.vector.reciprocal(rsum[:], ssum_ps[:1, :1])

                cun_ps = p1ps.tile([1, E], F32, space="PSUM")
                for half in range(2):
                    for i, rows in enumerate(rows3):
                        nc.tensor.matmul(cun_ps[:1, 512 * half:512 * (half + 1)],
                                         lhsT=escore[i][:rows, :1],
                                         rhs=ctx_sb[i][:rows, 512 * half:512 * (half + 1)],
                                         start=(i == 0), stop=(i == 2))
                c_sb = p1.tile([1, E], F32)
                nc.vector.tensor_scalar_mul(c_sb[:], cun_ps[:1, :], rsum[:1, :1])

                cT_ps = p1ps.tile([128, EC], F32, space="PSUM")
                for k in range(EC):
                    nc.tensor.transpose(out=cT_ps[:, k:k + 1],
                                        in_=c_sb[:1, 128 * k:128 * (k + 1)],
                                        identity=ident[:1, :1])
                nc.vector.tensor_copy(cT_bf[:], cT_ps[:])

                dxT_ps = p1ps.tile([128, T], F32, space="PSUM")
                for k in range(EC):
                    nc.tensor.transpose(out=dxT_ps[:, :],
                                        in_=decx_sb[:T, 128 * k:128 * (k + 1)],
                                        identity=ident[:T, :T])
                    nc.vector.tensor_copy(dxT_bf[:, k, :], dxT_ps[:, :])

            for rep in range(outer_reps):
                # ---------- phase 2: gic = W_ih[:, :E] @ c + biases ----------
                with tc.tile_pool(name=f"pwc{rep}", bufs=1) as pwc, \
                     tc.tile_pool(name=f"pwcps{rep}", bufs=1, space="PSUM") as pwcps:
                    wc_sb = pwc.tile([128, EC * CW], BF16)
                    for c in range(EC):
                        nc.sync.dma_start(wc_sb[:, c * CW:(c + 1) * CW],
                                          wc_d[:, c * CW:(c + 1) * CW])
                    wcv = wc_sb[:].rearrange("p (c j m) -> p c j m", c=EC, j=G)
                    gic_ps = pwcps.tile([128, 1024], F32, space="PSUM")
                    for c in range(EC):
                        for j in range(G):
                            nc.tensor.matmul(gic_ps[32 * j:32 * j + 1, 0:512],
                                             lhsT=cT_bf[:, c:c + 1],
                                             rhs=wcv[:, c, j, 0:512],
                                             start=(c == 0), stop=False,
                                             tile_position=(0, 32 * j))
                            nc.tensor.matmul(gic_ps[32 * j:32 * j + 1, 512:768],
                                             lhsT=cT_bf[:, c:c + 1],
                                             rhs=wcv[:, c, j, 512:768],
                                             start=(c == 0), stop=False,
                                             tile_position=(0, 32 * j))
                    for j in range(G):
                        nc.tensor.matmul(gic_ps[32 * j:32 * j + 1, 0:512],
                                         lhsT=ones_tall[32 * j:32 * j + 1, 0:1],
                                         rhs=bias_tall[32 * j:32 * j + 1, 0:512],
                                         start=False, stop=True,
                                         tile_position=(32 * j, 32 * j))
                        nc.tensor.matmul(gic_ps[32 * j:32 * j + 1, 512:768],
                                         lhsT=ones_tall[32 * j:32 * j + 1, 0:1],
                                         rhs=bias_tall[32 * j:32 * j + 1, 512:768],
                                         start=False, stop=True,
                                         tile_position=(32 * j, 32 * j))
                    for j in range(G):
                        if j % 2 == 0:
                            nc.scalar.copy(gic_tall[32 * j:32 * j + 1, :],
                                           gic_ps[32 * j:32 * j + 1, 0:RG])
                        else:
                            nc.vector.tensor_copy(gic_tall[32 * j:32 * j + 1, :],
                                                  gic_ps[32 * j:32 * j + 1, 0:RG])

                # ---------- phase 3: gi[t] = gic + W_ih[:, E:] @ x_t (batched) ----------
                with tc.tile_pool(name=f"pwx{rep}", bufs=1) as pwx, \
                     tc.tile_pool(name=f"pwxps{rep}", bufs=1, space="PSUM") as pwxps:
                    wx_sb = pwx.tile([128, EC * CW], BF16)
                    for c in range(EC):
                        nc.sync.dma_start(wx_sb[:, c * CW:(c + 1) * CW],
                                          wx_d[:, c * CW:(c + 1) * CW])
                    wxv = wx_sb[:].rearrange("p (c j m) -> p c j m", c=EC, j=G)
                    rzts = [pwxps.tile([T, 512], F32, space="PSUM", tag=f"grz{j}",
                                       name=f"grz{j}") for j in range(G)]
                    npts = [pwxps.tile([T, 256], F32, space="PSUM", tag=f"gn{j}",
                                       name=f"gn{j}") for j in range(G)]
                    for c in range(EC):
                        for j in range(G):
                            nc.tensor.matmul(rzts[j][:T, :], lhsT=dxT_bf[:, c, :],
                                             rhs=wxv[:, c, j, 0:512],
                                             start=(c == 0), stop=False)
                            nc.tensor.matmul(npts[j][:T, :], lhsT=dxT_bf[:, c, :],
                                             rhs=wxv[:, c, j, 512:768],
                                             start=(c == 0), stop=False)
                    for j in range(G):
                        nc.tensor.matmul(rzts[j][:T, :],
                                         lhsT=ones_tall[32 * j:32 * j + 1, :T],
                                         rhs=gic_tall[32 * j:32 * j + 1, 0:512],
                                         start=False, stop=True,
                                         tile_position=(32 * j, 0))
                        nc.vector.tensor_copy(girz[:, j, :], rzts[j][:T, :])
                        nc.tensor.matmul(npts[j][:T, :],
                                         lhsT=ones_tall[32 * j:32 * j + 1, :T],
                                         rhs=gic_tall[32 * j:32 * j + 1, 512:768],
                                         start=False, stop=True,
                                         tile_position=(32 * j, 0))
                        nc.vector.tensor_copy(gin65[:, 256 * j:256 * (j + 1)],
                                              npts[j][:T, :])

                # prefetch final-phase weights (after whh/wc/wx in program order)
                if rep == 0:
                    owt_sb = pp.tile([128, EC * VP], BF16)
                    outb_sb = pp.tile([1, VP], F32)
                    if do_final:
                        nc.sync.dma_start(owt_sb[:], owt_d[:])
                        nc.sync.dma_start(outb_sb[:], outb_d[:])

                # ---------- phase 4: Picard sweeps ----------
                # hprevT[:, c, 0] = h0 (sweeps 2+ read it; cols 1: from scans)
                nc.vector.tensor_copy(hprevT_v[:, :, 0:1], h_stat[:].unsqueeze(2))

                with tc.tile_pool(name=f"sw{rep}", bufs=4) as psw, \
                     tc.tile_pool(name=f"swg{rep}", bufs=2) as psg2, \
                     tc.tile_pool(name=f"swps{rep}", bufs=2, space="PSUM") as pps1, \
                     tc.tile_pool(name=f"swpsT{rep}", bufs=1, space="PSUM") as pps2:
                    # sweep-1 prologue: gh0 = W_hh @ h0 (col-tiled matvec, M=1).
                    # psg0 borrows the zT buffer (tag reuse; lifetimes disjoint).
                    psg0 = pps2.tile([128, 1024], F32, space="PSUM", tag="zT")
                    for c in range(EC):
                        for j in range(G):
                            nc.tensor.matmul(psg0[32 * j:32 * j + 1, 0:512],
                                             lhsT=h0bf[:, c:c + 1],
                                             rhs=whhv[:, c, j, 0:512],
                                             start=(c == 0), stop=(c == EC - 1),
                                             tile_position=(0, 32 * j))
                            nc.tensor.matmul(psg0[32 * j:32 * j + 1, 512:768],
                                             lhsT=h0bf[:, c:c + 1],
                                             rhs=whhv[:, c, j, 512:768],
                                             start=(c == 0), stop=(c == EC - 1),
                                             tile_position=(0, 32 * j))
                    for j in range(G):
                        if j % 2 == 0:
                            nc.scalar.copy(gh0[32 * j:32 * j + 1, :],
                                           psg0[32 * j:32 * j + 1, 0:RG])
                        else:
                            nc.vector.tensor_copy(gh0[32 * j:32 * j + 1, :],
                                                  psg0[32 * j:32 * j + 1, 0:RG])

                    for k in range(ksweeps):
                        first = (k == 0)
                        sgs, npres = [], []
                        # rz pass: 4 regions, gi fold + 8 K-chunks each
                        for j in range(G):
                            rz = pps1.tile([T, 512], F32, space="PSUM", tag="rz")
                            nc.tensor.matmul(rz[:T, :], lhsT=ident[:T, :T],
                                             rhs=girz[:, j, :], start=True, stop=False)
                            if first:
                                nc.tensor.matmul(rz[:T, :],
                                                 lhsT=ones_tall[32 * j:32 * j + 1, :T],
                                                 rhs=gh0[32 * j:32 * j + 1, 0:512],
                                                 start=False, stop=True,
                                                 tile_position=(32 * j, 0))
                            else:
                                for c in range(EC):
                                    nc.tensor.matmul(rz[:T, :],
                                                     lhsT=hprevT[:, c * TP:c * TP + T],
                                                     rhs=whhv[:, c, j, 0:512],
                                                     start=False, stop=(c == EC - 1))
                            sg = psw.tile([T, 512], F32, tag="sg")
                            nc.scalar.activation(sg[:], rz[:T, :], AF.Sigmoid)
                            sgs.append(sg)
                        # n pass
                        for j in range(G):
                            nps_j = pps1.tile([T, 256], F32, space="PSUM", tag="n")
                            if first:
                                nc.tensor.matmul(nps_j[:T, :],
                                                 lhsT=ones_tall[32 * j:32 * j + 1, :T],
                                                 rhs=gh0[32 * j:32 * j + 1, 512:768],
                                                 start=True, stop=True,
                                                 tile_position=(32 * j, 0))
                            else:
                                for c in range(EC):
                                    nc.tensor.matmul(nps_j[:T, :],
                                                     lhsT=hprevT[:, c * TP:c * TP + T],
                                                     rhs=whhv[:, c, j, 512:768],
                                                     start=(c == 0), stop=(c == EC - 1))
                            t1 = psg2.tile([T, 256], F32, tag="t1")
                            nc.vector.tensor_tensor(out=t1[:], in0=sgs[j][:, 0:256],
                                                    in1=nps_j[:T, :], op=ALU.mult)
                            npre = psw.tile([T, 256], F32, tag="npre")
                            nc.vector.tensor_tensor(out=npre[:], in0=t1[:],
                                                    in1=gin65[:, 256 * j:256 * (j + 1)],
                                                    op=ALU.add)
                            npres.append(npre)
                        # transposes into [128, (half, cc, t)] PSUM; chunk cc at
                        # column 512*(cc//4) + 65*(cc%4)
                        zT = pps2.tile([128, 1024], F32, space="PSUM", tag="zT")
                        npT = pps2.tile([128, 1024], F32, space="PSUM", tag="npT")
                        for cc in range(EC):
                            j, k2 = cc // 2, cc % 2
                            col = 512 * (cc // 4) + T * (cc % 4)
                            nc.tensor.transpose(out=zT[:, col:col + T],
                                                in_=sgs[j][:T, 256 + 128 * k2:256 + 128 * (k2 + 1)],
                                                identity=ident[:T, :T])
                            nc.tensor.transpose(out=npT[:, col:col + T],
                                                in_=npres[j][:T, 128 * k2:128 * (k2 + 1)],
                                                identity=ident[:T, :T])
                        nT = psg2.tile([128, 1024], F32, tag="nT")
                        wsb = psg2.tile([128, 1024], F32, tag="wsb")
                        for h in range(2):
                            s = slice(512 * h, 512 * h + 4 * T)
                            nc.scalar.activation(nT[:, s], npT[:, s], AF.Tanh)
                            # (z - 1) * n
                            nc.vector.scalar_tensor_tensor(out=wsb[:, s], in0=zT[:, s],
                                                           scalar=1.0, in1=nT[:, s],
                                                           op0=ALU.subtract, op1=ALU.mult)
                        for cc in range(EC):
                            col = 512 * (cc // 4) + T * (cc % 4)
                            # state = z*state - (z-1)*n; writes h_1..h_65 into
                            # slots (cc, 1..65); next sweep's lhsT reads (cc, 0..64)
                            nc.vector.tensor_tensor_scan(
                                out=hprevT[:, cc * TP + 1:cc * TP + 1 + T],
                                data0=zT[:, col:col + T], data1=wsb[:, col:col + T],
                                initial=h_stat[:, cc:cc + 1],
                                op0=ALU.mult, op1=ALU.subtract)

                # ---------- phase 5: logits = relu(H) @ out_w^T + out_b ----------
                nc.scalar.activation(htf[:].rearrange("p (c t) -> p c t", c=EC),
                                     hprevT_v[:, :, 1:TP], AF.Relu)
                owtv = owt_sb[:].rearrange("p (c v) -> p c v", c=EC)
                htv = htf[:].rearrange("p (c t) -> p c t", c=EC)
                if not do_final and rep == 0:
                    nc.sync.dma_start(out_d[0:T, 0:T], ones_tall[:T, :T])
                with tc.tile_pool(name=f"fin{rep}", bufs=2) as pf, \
                     tc.tile_pool(name=f"finps{rep}", bufs=2, space="PSUM") as pfps:
                    for vb in range(VP // 512 if do_final else 0):
                        ops = pfps.tile([T, 512], F32, space="PSUM", tag="ops")
                        for c in range(EC):
                            nc.tensor.matmul(ops[:T, :], lhsT=htv[:, c, :],
                                             rhs=owtv[:, c, 512 * vb:512 * (vb + 1)],
                                             start=(c == 0), stop=False)
                        nc.tensor.matmul(ops[:T, :], lhsT=ones_tall[:1, :T],
                                         rhs=outb_sb[:1, 512 * vb:512 * (vb + 1)],
                                         start=False, stop=True)
                        osb = pf.tile([T, 512], F32, tag="osb")
                        if vb % 2 == 0:
                            nc.vector.tensor_copy(osb[:], ops[:T, :])
                        else:
                            nc.scalar.copy(osb[:], ops[:T, :])
                        nc.sync.dma_start(out_d[:, 512 * vb:512 * (vb + 1)], osb[:])

    nc.compile()
    return nc


def _prep_inputs(inp):
    idx_enc = np.concatenate([inp["input_diagnosis"], inp["input_procedure"],
                              inp["input_medicine"]]).astype(np.int64)
    tokens = np.concatenate([np.array([V0], np.int64),
                             inp["dec_tokens"].astype(np.int64)])
    enc_emb = np.asarray(inp["enc_emb"], np.float32)
    dec_emb = np.asarray(inp["dec_emb"], np.float32)

    ctx = np.ascontiguousarray(enc_emb[idx_enc])                       # [320, 1024]
    decx = np.ascontiguousarray(dec_emb[tokens])                       # [65, 1024]
    we = np.ascontiguousarray(np.asarray(inp["attn_w"], np.float32)[0, E:]).reshape(1, E)

    w_ih = np.asarray(inp["gru_w_ih"], np.float32)                     # [3072, 2048]
    w_hh = np.asarray(inp["gru_w_hh"], np.float32)                     # [3072, 1024]
    b_ih = np.asarray(inp["gru_b_ih"], np.float32)
    b_hh = np.asarray(inp["gru_b_hh"], np.float32)
    assert not np.any(b_hh[2 * E:]), "nonzero b_hh n-gate not supported on device"

    whh_arr = _arrange_w(w_hh).astype(NP_BF16)                         # [128, 24576]
    wc_arr = _arrange_w(np.ascontiguousarray(w_ih[:, :E])).astype(NP_BF16)
    wx_arr = _arrange_w(np.ascontiguousarray(w_ih[:, E:])).astype(NP_BF16)
    bias = b_ih.copy()
    bias[:2 * E] += b_hh[:2 * E]
    bias_arr = _bias_tall(bias)                                        # [128, 768] f32

    out_w = np.asarray(inp["out_w"], np.float32)
    out_b = np.asarray(inp["out_b"], np.float32)
    owp = np.zeros((NCORES * VP, E), np.float32)
    owp[:V] = out_w
    obp = np.zeros(NCORES * VP, np.float32)
    obp[:V] = out_b

    base = {"ctx": ctx, "decx": decx, "we": we, "whh": whh_arr,
            "wc": wc_arr, "wx": wx_arr, "bias": bias_arr}
    in_maps = []
    for i in range(NCORES):
        s = owp[i * VP:(i + 1) * VP]                                   # [4096, 1024]
        owt = np.ascontiguousarray(
            s.reshape(VP, EC, 128).transpose(2, 1, 0)).astype(NP_BF16).reshape(128, EC * VP)
        m = dict(base)
        m["owt"] = owt
        m["outb"] = np.ascontiguousarray(obp[i * VP:(i + 1) * VP]).reshape(1, VP)
        in_maps.append(m)
    return in_maps


def kernel(**inputs):
    if "nc" not in _CACHE:
        _CACHE["nc"] = build_program()
    nc = _CACHE["nc"]
    in_maps = _prep_inputs({k: np.asarray(v) for k, v in inputs.items()})
    res = run_bass_kernel_spmd(nc, in_maps, core_ids=list(range(NCORES)))
    slices = [res.results[i]["out"] for i in range(NCORES)]            # each [65, 4096]
    logits = np.concatenate(slices, axis=1)[:, :V]
    return np.ascontiguousarray(logits.astype(np.float32))


# revision 4
# speedup vs baseline: 41.6996x; 1.4959x over previous
"""Trainium2 Bass kernel for nn_LEAP_74371653697613 (GRU decoder w/ additive attention).

v2: Picard-sweep formulation of the recurrence.

Structure exploited:
  - softmax(ctx_score + h.w_h) == softmax(ctx_score): attention weights constant
    across steps -> context c computed once.
  - gi_t = W_ih @ [c; x_t] + b_ih precomputed batched (teacher forcing).
  - The h-recurrence h_t = (1-z_t) n_t + z_t h_{t-1} is solved by fixed-point
    iteration over the WHOLE sequence: each sweep computes gh = W_hh @ H_prev as
    ONE batched [65,3072] matmul (W_hh streamed once per sweep instead of once
    per step), gates batched, then the exact sequential structure is restored by
    the DVE prefix-scan  state = (z_t * state) - (z_t - 1)*n_t  per hidden chunk.
    Contraction ~0.3x/sweep; KS sweeps reach the bf16 noise floor (~2.6e-3 rel).
  - Sweep 1 has H_prev === h0, so its gh is a single col-tiled matvec W_hh @ h0
    broadcast across t (cheap) instead of the batched matmul.
  - logits = relu(H) @ out_w^T batched at the end, vocab-sharded 8 ways
    (each core owns a 4096-row slice of out_w); out_w prefetched during sweeps.

outer_reps repeats phases 2-5 inside one NEFF for slope-based timing; the
shipped kernel uses outer_reps=1.
"""
import os
import sys
import numpy as np

for _p in ("/opt/trn_rl_repo", "/root/.axon_site/_ro/trn_rl_repo"):
    if os.path.isdir(_p) and _p not in sys.path:
        sys.path.insert(0, _p)

import concourse.bass as bass
import concourse.bacc as bacc
import concourse.tile as tile
import concourse.mybir as mybir
from concourse.bass_utils import run_bass_kernel_spmd
from concourse.masks import make_identity

F32 = mybir.dt.float32
BF16 = mybir.dt.bfloat16
AF = mybir.ActivationFunctionType
ALU = mybir.AluOpType
NP_BF16 = mybir.dt.np(BF16)

E = 1024          # emb dim
EC = 8            # E / 128 chunks
T = 65            # decode steps (1 SOS + 64)
L = 320           # context rows (128 + 64 + 128)
V0 = 32000
V = V0 + 2        # 32002
NCORES = 8
VP = 4096         # per-core padded vocab slice (8 * 4096 = 32768 >= 32002)
G = 4             # gate regions (each 256 hidden units x 3 gates)
RG = 768          # region width (3 gates x 256)
CW = G * RG       # per-K-chunk weight width (3072)
KS = 6            # Picard sweeps (incl. the specialized first sweep)
TP = 66           # padded t-stride in hprevT: slot 0 = h0, slots 1..65 = scan out

_CACHE = {}


def _arrange_w(w):
    """[3072, 1024] -> [128, 8*4*768]: out[p, ((c*4)+j)*768 + g*256+mm]
    = w[g*1024 + j*256 + mm, c*128 + p]."""
    x = w.reshape(3, 4, 256, EC, 128)            # g, j, mm, c, p
    x = np.transpose(x, (4, 3, 1, 0, 2))         # p, c, j, g, mm
    return np.ascontiguousarray(x).reshape(128, EC * CW)


def _bias_tall(b_rzn):
    x = b_rzn.reshape(3, 4, 256)                 # g, j, mm
    x = np.transpose(x, (1, 0, 2)).reshape(4, RG)  # j, (g mm)
    out = np.zeros((128, RG), np.float32)
    out[::32, :] = x
    return out


def build_program(ksweeps=KS, do_final=True, outer_reps=1):
    nc = bacc.Bacc("TRN2", target_bir_lowering=False, debug=False, num_devices=NCORES)

    ctx_d = nc.dram_tensor("ctx", [L, E], F32, kind="ExternalInput").ap()
    decx_d = nc.dram_tensor("decx", [T, E], F32, kind="ExternalInput").ap()
    we_d = nc.dram_tensor("we", [1, E], F32, kind="ExternalInput").ap()
    whh_d = nc.dram_tensor("whh", [128, EC * CW], BF16, kind="ExternalInput").ap()
    wc_d = nc.dram_tensor("wc", [128, EC * CW], BF16, kind="ExternalInput").ap()
    wx_d = nc.dram_tensor("wx", [128, EC * CW], BF16, kind="ExternalInput").ap()
    bias_d = nc.dram_tensor("bias", [128, RG], F32, kind="ExternalInput").ap()
    owt_d = nc.dram_tensor("owt", [128, EC * VP], BF16, kind="ExternalInput").ap()
    outb_d = nc.dram_tensor("outb", [1, VP], F32, kind="ExternalInput").ap()
    out_d = nc.dram_tensor("out", [T, VP], BF16, kind="ExternalOutput").ap()

    with tile.TileContext(nc) as tc:
        with tc.tile_pool(name="persist", bufs=1) as pp:
            # ---------- persistent constants ----------
            whh = pp.tile([128, EC * CW], BF16)
            for c in range(EC):
                nc.sync.dma_start(whh[:, c * CW:(c + 1) * CW],
                                  whh_d[:, c * CW:(c + 1) * CW])

            ident = pp.tile([128, 128], F32)
            make_identity(nc, ident[:])
            ident_bf = pp.tile([128, 128], BF16)
            nc.vector.tensor_copy(ident_bf[:], ident[:])

            ones_tall = pp.tile([128, T], F32)
            nc.gpsimd.memset(ones_tall[:], 1.0)

            bias_tall = pp.tile([128, RG], F32)
            nc.sync.dma_start(bias_tall[:], bias_d[:])

            girz = pp.tile([T, G, 512], BF16)      # gi rz-part, partition = t
            gin65 = pp.tile([T, 1024], F32)        # gi n-part, partition = t
            gic_tall = pp.tile([128, RG], F32)     # const part of gi, rows at 32j
            h_stat = pp.tile([128, EC], F32)       # h0 chunks (scan initial)
            h0bf = pp.tile([128, EC], BF16)
            gh0 = pp.tile([128, RG], F32)          # W_hh @ h0, region rows at 32j
            hprevT = pp.tile([128, EC * TP], BF16)  # slot (c,0)=h0; (c,1..65)=h_1..h_65
            htf = pp.tile([128, EC * T], BF16)      # relu(h_1..h_65)
            cT_bf = pp.tile([128, EC], BF16)
            dxT_bf = pp.tile([128, EC, T], BF16)

            # h0 = dec_emb[SOS] = decx row 0 in stationary layout
            nc.sync.dma_start(h_stat[:], decx_d[0:1, :].rearrange("o (c p) -> (o p) c", p=128))
            nc.vector.tensor_copy(h0bf[:], h_stat[:])

            whhv = whh[:].rearrange("p (c j m) -> p c j m", c=EC, j=G)
            hprevT_v = hprevT[:].rearrange("p (c t) -> p c t", c=EC)

            # ---------- phase 1: attention (constant across steps) ----------
            with tc.tile_pool(name="ph1", bufs=1) as p1, \
                 tc.tile_pool(name="ph1ps", bufs=1, space="PSUM") as p1ps:
                ones_col = p1.tile([128, 1], F32)
                nc.gpsimd.memset(ones_col[:], 1.0)
                ones_row = p1.tile([1, 128], F32)
                nc.gpsimd.memset(ones_row[:], 1.0)
                we_sb = p1.tile([1, E], F32)
                nc.sync.dma_start(we_sb[:], we_d[:])
                rows3 = (128, 128, 64)
                ctx_sb = []
                for i, rows in enumerate(rows3):
                    t_ = p1.tile([128, E], F32, tag=f"ctx{i}")
                    nc.sync.dma_start(t_[:rows, :], ctx_d[128 * i:128 * i + rows, :])
                    ctx_sb.append(t_)
                decx_sb = p1.tile([T, E], F32)
                nc.sync.dma_start(decx_sb[:], decx_d[:])

                werep_ps = p1ps.tile([128, E], F32, space="PSUM")
                for half in range(2):
                    nc.tensor.matmul(werep_ps[:, 512 * half:512 * (half + 1)],
                                     lhsT=ones_row[:1, :],
                                     rhs=we_sb[:1, 512 * half:512 * (half + 1)],
                                     start=True, stop=True)
                werep = p1.tile([128, E], F32)
                nc.vector.tensor_copy(werep[:], werep_ps[:])

                scratch = p1.tile([128, E], F32)
                escore = [p1.tile([128, 1], F32, tag=f"esc{i}", name=f"esc{i}")
                          for i in range(3)]
                for i, rows in enumerate(rows3):
                    sc = p1.tile([128, 1], F32, tag=f"sc{i}")
                    nc.vector.tensor_tensor(out=scratch[:rows, :],
                                            in0=ctx_sb[i][:rows, :],
                                            in1=werep[:rows, :], op=ALU.mult)
                    nc.vector.tensor_reduce(out=sc[:rows, :], in_=scratch[:rows, :],
                                            axis=mybir.AxisListType.X, op=ALU.add)
                    nc.scalar.activation(escore[i][:rows, :], sc[:rows, :], AF.Exp)
                ssum_ps = p1ps.tile([1, 1], F32, space="PSUM")
                for i, rows in enumerate(rows3):
                    nc.tensor.matmul(ssum_ps[:1, :1], lhsT=escore[i][:rows, :1],
                                     rhs=ones_col[:rows, :1],
                                     start=(i == 0), stop=(i == 2))
                rsum = p1.tile([1, 1], F32)
                nc.vector.reciprocal(rsum[:], ssum_ps[:1, :1])

                cun_ps = p1ps.tile([1, E], F32, space="PSUM")
                for half in range(2):
                    for i, rows in enumerate(rows3):
                        nc.tensor.matmul(cun_ps[:1, 512 * half:512 * (half + 1)],
                                         lhsT=escore[i][:rows, :1],
                                         rhs=ctx_sb[i][:rows, 512 * half:512 * (half + 1)],
                                         start=(i == 0), stop=(i == 2))
                c_sb = p1.tile([1, E], F32)
                nc.vector.tensor_scalar_mul(c_sb[:], cun_ps[:1, :], rsum[:1, :1])

                cT_ps = p1ps.tile([128, EC], F32, space="PSUM")
                for k in range(EC):
                    nc.tensor.transpose(out=cT_ps[:, k:k + 1],
                                        in_=c_sb[:1, 128 * k:128 * (k + 1)],
                                        identity=ident[:1, :1])
                nc.vector.tensor_copy(cT_bf[:], cT_ps[:])

                dxT_ps = p1ps.tile([128, T], F32, space="PSUM")
                for k in range(EC):
                    nc.tensor.transpose(out=dxT_ps[:, :],
                                        in_=decx_sb[:T, 128 * k:128 * (k + 1)],
                                        identity=ident[:T, :T])
                    nc.vector.tensor_copy(dxT_bf[:, k, :], dxT_ps[:, :])

            for rep in range(outer_reps):
                # ---------- phase 2: gic = W_ih[:, :E] @ c + biases ----------
                with tc.tile_pool(name=f"pwc{rep}", bufs=1) as pwc, \
                     tc.tile_pool(name=f"pwcps{rep}", bufs=1, space="PSUM") as pwcps:
                    wc_sb = pwc.tile([128, EC * CW], BF16)
                    for c in range(EC):
                        nc.sync.dma_start(wc_sb[:, c * CW:(c + 1) * CW],
                                          wc_d[:, c * CW:(c + 1) * CW])
                    wcv = wc_sb[:].rearrange("p (c j m) -> p c j m", c=EC, j=G)
                    gic_ps = pwcps.tile([128, 1024], F32, space="PSUM")
                    for c in range(EC):
                        for j in range(G):
                            nc.tensor.matmul(gic_ps[32 * j:32 * j + 1, 0:512],
                                             lhsT=cT_bf[:, c:c + 1],
                                             rhs=wcv[:, c, j, 0:512],
                                             start=(c == 0), stop=False,
                                             tile_position=(0, 32 * j))
                            nc.tensor.matmul(gic_ps[32 * j:32 * j + 1, 512:768],
                                             lhsT=cT_bf[:, c:c + 1],
                                             rhs=wcv[:, c, j, 512:768],
                                             start=(c == 0), stop=False,
                                             tile_position=(0, 32 * j))
                    for j in range(G):
                        nc.tensor.matmul(gic_ps[32 * j:32 * j + 1, 0:512],
                                         lhsT=ones_tall[32 * j:32 * j + 1, 0:1],
                                         rhs=bias_tall[32 * j:32 * j + 1, 0:512],
                                         start=False, stop=True,
                                         tile_position=(32 * j, 32 * j))
                        nc.tensor.matmul(gic_ps[32 * j:32 * j + 1, 512:768],
                                         lhsT=ones_tall[32 * j:32 * j + 1, 0:1],
                                         rhs=bias_tall[32 * j:32 * j + 1, 512:768],
                                         start=False, stop=True,
                                         tile_position=(32 * j, 32 * j))
                    for j in range(G):
                        if j % 2 == 0:
                            nc.scalar.copy(gic_tall[32 * j:32 * j + 1, :],
                                           gic_ps[32 * j:32 * j + 1, 0:RG])
                        else:
                            nc.vector.tensor_copy(gic_tall[32 * j:32 * j + 1, :],
                                                  gic_ps[32 * j:32 * j + 1, 0:RG])

                # ---------- phase 3: gi[t] = gic + W_ih[:, E:] @ x_t (batched) ----------
                with tc.tile_pool(name=f"pwx{rep}", bufs=1) as pwx, \
                     tc.tile_pool(name=f"pwxps{rep}", bufs=1, space="PSUM") as pwxps:
                    wx_sb = pwx.tile([128, EC * CW], BF16)
                    for c in range(EC):
                        nc.sync.dma_start(wx_sb[:, c * CW:(c + 1) * CW],
                                          wx_d[:, c * CW:(c + 1) * CW])
                    wxv = wx_sb[:].rearrange("p (c j m) -> p c j m", c=EC, j=G)
                    rzts = [pwxps.tile([T, 512], F32, space="PSUM", tag=f"grz{j}",
                                       name=f"grz{j}") for j in range(G)]
                    npts = [pwxps.tile([T, 256], F32, space="PSUM", tag=f"gn{j}",
                                       name=f"gn{j}") for j in range(G)]
                    for c in range(EC):
                        for j in range(G):
                            nc.tensor.matmul(rzts[j][:T, :], lhsT=dxT_bf[:, c, :],
                                             rhs=wxv[:, c, j, 0:512],
                                             start=(c == 0), stop=False)
                            nc.tensor.matmul(npts[j][:T, :], lhsT=dxT_bf[:, c, :],
                                             rhs=wxv[:, c, j, 512:768],
                                             start=(c == 0), stop=False)
                    for j in range(G):
                        nc.tensor.matmul(rzts[j][:T, :],
                                         lhsT=ones_tall[32 * j:32 * j + 1, :T],
                                         rhs=gic_tall[32 * j:32 * j + 1, 0:512],
                                         start=False, stop=True,
                                         tile_position=(32 * j, 0))
                        nc.vector.tensor_copy(girz[:, j, :], rzts[j][:T, :])
                        nc.tensor.matmul(npts[j][:T, :],
                                         lhsT=ones_tall[32 * j:32 * j + 1, :T],
                                         rhs=gic_tall[32 * j:32 * j + 1, 512:768],
                                         start=False, stop=True,
                                         tile_position=(32 * j, 0))
                        nc.vector.tensor_copy(gin65[:, 256 * j:256 * (j + 1)],
                                              npts[j][:T, :])

                # prefetch final-phase weights (after whh/wc/wx in program order)
                if rep == 0:
                    owt_sb = pp.tile([128, EC * VP], BF16)
                    outb_sb = pp.tile([1, VP], F32)
                    if do_final:
                        nc.sync.dma_start(owt_sb[:], owt_d[:])
                        nc.sync.dma_start(outb_sb[:], outb_d[:])

                # ---------- phase 4: Picard sweeps ----------
                # hprevT[:, c, 0] = h0 (sweeps 2+ read it; cols 1: from scans)
                nc.vector.tensor_copy(hprevT_v[:, :, 0:1], h_stat[:].unsqueeze(2))

                with tc.tile_pool(name=f"sw{rep}", bufs=4) as psw, \
                     tc.tile_pool(name=f"swg{rep}", bufs=2) as psg2, \
                     tc.tile_pool(name=f"swps{rep}", bufs=2, space="PSUM") as pps1, \
                     tc.tile_pool(name=f"swpsT{rep}", bufs=1, space="PSUM") as pps2:
                    # sweep-1 prologue: gh0 = W_hh @ h0 (col-tiled matvec, M=1).
                    # psg0 borrows the zT buffer (tag reuse; lifetimes disjoint).
                    psg0 = pps2.tile([128, 1024], F32, space="PSUM", tag="zT")
                    for c in range(EC):
                        for j in range(G):
                            nc.tensor.matmul(psg0[32 * j:32 * j + 1, 0:512],
                                             lhsT=h0bf[:, c:c + 1],
                                             rhs=whhv[:, c, j, 0:512],
                                             start=(c == 0), stop=(c == EC - 1),
                                             tile_position=(0, 32 * j))
                            nc.tensor.matmul(psg0[32 * j:32 * j + 1, 512:768],
                                             lhsT=h0bf[:, c:c + 1],
                                             rhs=whhv[:, c, j, 512:768],
                                             start=(c == 0), stop=(c == EC - 1),
                                             tile_position=(0, 32 * j))
                    for j in range(G):
                        if j % 2 == 0:
                            nc.scalar.copy(gh0[32 * j:32 * j + 1, :],
                                           psg0[32 * j:32 * j + 1, 0:RG])
                        else:
                            nc.vector.tensor_copy(gh0[32 * j:32 * j + 1, :],
                                                  psg0[32 * j:32 * j + 1, 0:RG])

                    for k in range(ksweeps):
                        first = (k == 0)
                        sgs, npres = [], []
                        # rz pass: 4 regions, gi fold + 8 K-chunks each
                        for j in range(G):
                            rz = pps1.tile([T, 512], F32, space="PSUM", tag="rz")
                            nc.tensor.matmul(rz[:T, :], lhsT=ident_bf[:T, :T],
                                             rhs=girz[:, j, :], start=True, stop=False)
                            if first:
                                nc.tensor.matmul(rz[:T, :],
                                                 lhsT=ones_tall[32 * j:32 * j + 1, :T],
                                                 rhs=gh0[32 * j:32 * j + 1, 0:512],
                                                 start=False, stop=True,
                                                 tile_position=(32 * j, 0))
                            else:
                                for c in range(EC):
                                    nc.tensor.matmul(rz[:T, :],
                                                     lhsT=hprevT[:, c * TP:c * TP + T],
                                                     rhs=whhv[:, c, j, 0:512],
                                                     start=False, stop=(c == EC - 1))
                            sg = psw.tile([T, 512], F32, tag="sg")
                            nc.scalar.activation(sg[:], rz[:T, :], AF.Sigmoid)
                            sgs.append(sg)
                        # n pass
                        for j in range(G):
                            nps_j = pps1.tile([T, 256], F32, space="PSUM", tag="n")
                            if first:
                                nc.tensor.matmul(nps_j[:T, :],
                                                 lhsT=ones_tall[32 * j:32 * j + 1, :T],
                                                 rhs=gh0[32 * j:32 * j + 1, 512:768],
                                                 start=True, stop=True,
                                                 tile_position=(32 * j, 0))
                            else:
                                for c in range(EC):
                                    nc.tensor.matmul(nps_j[:T, :],
                                                     lhsT=hprevT[:, c * TP:c * TP + T],
                                                     rhs=whhv[:, c, j, 512:768],
                                                     start=(c == 0), stop=(c == EC - 1))
                            t1 = psg2.tile([T, 256], F32, tag="t1")
                            nc.vector.tensor_tensor(out=t1[:], in0=sgs[j][:, 0:256],
                                                    in1=nps_j[:T, :], op=ALU.mult)
                            npre = psw.tile([T, 256], F32, tag="npre")
                            nc.vector.tensor_tensor(out=npre[:], in0=t1[:],
                                                    in1=gin65[:, 256 * j:256 * (j + 1)],
                                                    op=ALU.add)
                            npres.append(npre)
                        # transposes into [128, (half, cc, t)] PSUM; chunk cc at
                        # column 512*(cc//4) + 65*(cc%4)
                        zT = pps2.tile([128, 1024], F32, space="PSUM", tag="zT")
                        npT = pps2.tile([128, 1024], F32, space="PSUM", tag="npT")
                        for cc in range(EC):
                            j, k2 = cc // 2, cc % 2
                            col = 512 * (cc // 4) + T * (cc % 4)
                            nc.tensor.transpose(out=zT[:, col:col + T],
                                                in_=sgs[j][:T, 256 + 128 * k2:256 + 128 * (k2 + 1)],
                                                identity=ident[:T, :T])
                            nc.tensor.transpose(out=npT[:, col:col + T],
                                                in_=npres[j][:T, 128 * k2:128 * (k2 + 1)],
                                                identity=ident[:T, :T])
                        nT = psg2.tile([128, 1024], F32, tag="nT")
                        wsb = psg2.tile([128, 1024], F32, tag="wsb")
                        for h in range(2):
                            s = slice(512 * h, 512 * h + 4 * T)
                            nc.scalar.activation(nT[:, s], npT[:, s], AF.Tanh)
                            # (z - 1) * n
                            nc.vector.scalar_tensor_tensor(out=wsb[:, s], in0=zT[:, s],
                                                           scalar=1.0, in1=nT[:, s],
                                                           op0=ALU.subtract, op1=ALU.mult)
                        for cc in range(EC):
                            col = 512 * (cc // 4) + T * (cc % 4)
                            # state = z*state - (z-1)*n; writes h_1..h_65 into
                            # slots (cc, 1..65); next sweep's lhsT reads (cc, 0..64)
                            nc.vector.tensor_tensor_scan(
                                out=hprevT[:, cc * TP + 1:cc * TP + 1 + T],
                                data0=zT[:, col:col + T], data1=wsb[:, col:col + T],
                                initial=h_stat[:, cc:cc + 1],
                                op0=ALU.mult, op1=ALU.subtract)

                # ---------- phase 5: logits = relu(H) @ out_w^T + out_b ----------
                nc.scalar.activation(htf[:].rearrange("p (c t) -> p c t", c=EC),
                                     hprevT_v[:, :, 1:TP], AF.Relu)
                owtv = owt_sb[:].rearrange("p (c v) -> p c v", c=EC)
                htv = htf[:].rearrange("p (c t) -> p c t", c=EC)
                if not do_final and rep == 0:
                    nc.sync.dma_start(out_d[0:T, 0:T], htf[:T, 0:T])
                with tc.tile_pool(name=f"fin{rep}", bufs=2) as pf, \
                     tc.tile_pool(name=f"finps{rep}", bufs=2, space="PSUM") as pfps:
                    for vb in range(VP // 512 if do_final else 0):
                        ops = pfps.tile([T, 512], F32, space="PSUM", tag="ops")
                        for c in range(EC):
                            nc.tensor.matmul(ops[:T, :], lhsT=htv[:, c, :],
                                             rhs=owtv[:, c, 512 * vb:512 * (vb + 1)],
                                             start=(c == 0), stop=False)
                        nc.tensor.matmul(ops[:T, :], lhsT=ones_tall[:1, :T],
                                         rhs=outb_sb[:1, 512 * vb:512 * (vb + 1)],
                                         start=False, stop=True)
                        osb = pf.tile([T, 512], BF16, tag="osb")
                        if vb % 2 == 0:
                            nc.vector.tensor_copy(osb[:], ops[:T, :])
                        else:
                            nc.scalar.copy(osb[:], ops[:T, :])
                        nc.sync.dma_start(out_d[:, 512 * vb:512 * (vb + 1)], osb[:])

    nc.compile()
    return nc


def _prep_inputs(inp):
    idx_enc = np.concatenate([inp["input_diagnosis"], inp["input_procedure"],
                              inp["input_medicine"]]).astype(np.int64)
    tokens = np.concatenate([np.array([V0], np.int64),
                             inp["dec_tokens"].astype(np.int64)])
    enc_emb = np.asarray(inp["enc_emb"], np.float32)
    dec_emb = np.asarray(inp["dec_emb"], np.float32)

    ctx = np.ascontiguousarray(enc_emb[idx_enc])                       # [320, 1024]
    decx = np.ascontiguousarray(dec_emb[tokens])                       # [65, 1024]
    we = np.ascontiguousarray(np.asarray(inp["attn_w"], np.float32)[0, E:]).reshape(1, E)

    w_ih = np.asarray(inp["gru_w_ih"], np.float32)                     # [3072, 2048]
    w_hh = np.asarray(inp["gru_w_hh"], np.float32)                     # [3072, 1024]
    b_ih = np.asarray(inp["gru_b_ih"], np.float32)
    b_hh = np.asarray(inp["gru_b_hh"], np.float32)
    assert not np.any(b_hh[2 * E:]), "nonzero b_hh n-gate not supported on device"

    whh_arr = _arrange_w(w_hh).astype(NP_BF16)                         # [128, 24576]
    wc_arr = _arrange_w(np.ascontiguousarray(w_ih[:, :E])).astype(NP_BF16)
    wx_arr = _arrange_w(np.ascontiguousarray(w_ih[:, E:])).astype(NP_BF16)
    bias = b_ih.copy()
    bias[:2 * E] += b_hh[:2 * E]
    bias_arr = _bias_tall(bias)                                        # [128, 768] f32

    out_w = np.asarray(inp["out_w"], np.float32)
    out_b = np.asarray(inp["out_b"], np.float32)
    owp = np.zeros((NCORES * VP, E), np.float32)
    owp[:V] = out_w
    obp = np.zeros(NCORES * VP, np.float32)
    obp[:V] = out_b

    base = {"ctx": ctx, "decx": decx, "we": we, "whh": whh_arr,
            "wc": wc_arr, "wx": wx_arr, "bias": bias_arr}
    in_maps = []
    for i in range(NCORES):
        s = owp[i * VP:(i + 1) * VP]                                   # [4096, 1024]
        owt = np.ascontiguousarray(
            s.reshape(VP, EC, 128).transpose(2, 1, 0)).astype(NP_BF16).reshape(128, EC * VP)
        m = dict(base)
        m["owt"] = owt
        m["outb"] = np.ascontiguousarray(obp[i * VP:(i + 1) * VP]).reshape(1, VP)
        in_maps.append(m)
    return in_maps


def kernel(**inputs):
    if "nc" not in _CACHE:
        _CACHE["nc"] = build_program()
    nc = _CACHE["nc"]
    in_maps = _prep_inputs({k: np.asarray(v) for k, v in inputs.items()})
    res = run_bass_kernel_spmd(nc, in_maps, core_ids=list(range(NCORES)))
    slices = [res.results[i]["out"] for i in range(NCORES)]            # each [65, 4096]
    logits = np.concatenate(slices, axis=1)[:, :V]
    return np.ascontiguousarray(logits.astype(np.float32))
